# revision 37
# baseline (speedup 1.0000x reference)
"""MHA kernel builder for TRN2 (per-core SPMD program) + host prep.

Problem: out = X + MHA(RMSNorm(X)) where Q=K=V=(RMSNorm(X)@Wq.T+b), rope,
causal softmax, Wo projection. Sharding: batch(2) x head-groups(4) over 8
cores; each core computes a partial of out[b] (its 4 heads through Wo);
host sums partials + bias + residual.

v2: fp16 everywhere on the PE path, software-pipelined attention emission
(scores/exp of block i interleaved with dn/AV of block i-1 and Wo of the
previous q-chunk), row reciprocals via the fast custom-DVE op, fp16 output.
"""
import math
import itertools
import numpy as np
from contextlib import ExitStack

import concourse.bass as bass
import concourse.mybir as mybir
import concourse.tile as tile

F32 = mybir.dt.float32
F32R = mybir.dt.float32r
F16 = mybir.dt.float16
BF16 = mybir.dt.bfloat16

EPS = float(np.finfo(np.float32).eps)
ROPE_BASE = 10000.0

_ctr = itertools.count()


def legalize_sync_waits(nc, max_waits=1):
    """This walrus accepts at most one sync-wait per instruction; hoist
    excess waits onto same-engine NOPs inserted just before."""
    n_fixed = 0
    for f in nc.m.functions:
        for bb in f.blocks:
            insts = bb.instructions
            out = []
            dirty = False
            for inst in insts:
                si = getattr(inst, "sync_info", None)
                if si is not None and si.on_wait and len(si.on_wait) > max_waits:
                    waits = list(si.on_wait)
                    for w in waits[:-max_waits]:
                        nop = mybir.InstNoOp(
                            name=f"I-syncfix-{next(_ctr)}", engine=inst.engine
                        )
                        nop.sync_info = mybir.SyncInfo(on_wait=[w], on_update=[])
                        nc.register_instruction(nop, overwrite=True)
                        out.append(nop)
                    inst.sync_info = mybir.SyncInfo(
                        on_wait=waits[-max_waits:], on_update=list(si.on_update or [])
                    )
                    dirty = True
                    n_fixed += 1
                out.append(inst)
            if dirty:
                bb.instructions = out
    return n_fixed


def build_core(S=2048, D=2048, NHL=4, DK=128, SHIFT=10.0, debug=False):
    """Emit the per-core program. Returns nc. All cores run this same NEFF
    with different input data."""
    assert S % 512 == 0 and D % 128 == 0 and DK == 128
    SK = S // 512     # 512-wide seq chunks
    KT = D // 128     # contraction tiles for projections
    ST = S // 128     # 128-wide seq tiles
    ML = NHL * DK     # local model width (q columns this core owns)

    nc = bass.Bass("TRN2", num_devices=8)
    dXT = nc.dram_tensor("XT", [D, S], F16, kind="ExternalInput")
    dWQT = nc.dram_tensor("WQT", [D, ML], F16, kind="ExternalInput")
    dWOT = nc.dram_tensor("WOT", [ML, D], F16, kind="ExternalInput")
    dQB = nc.dram_tensor("QB", [128, NHL], F32, kind="ExternalInput")
    dCOSA = nc.dram_tensor("COSA", [DK, S], F16, kind="ExternalInput")
    dSINA = nc.dram_tensor("SINA", [DK, S], F16, kind="ExternalInput")
    dTRI = nc.dram_tensor("TRI", [128, 128], F16, kind="ExternalInput")
    dONEC = nc.dram_tensor("ONEC", [128, 1], F32, kind="ExternalInput")
    dONER = nc.dram_tensor("ONER", [1, 128], F32, kind="ExternalInput")
    dIDT = nc.dram_tensor("IDT", [128, 128], F16, kind="ExternalInput")
    dOUT = nc.dram_tensor("OUTP", [S, D], F16, kind="ExternalOutput")
    if debug:
        dDBG_Q = nc.dram_tensor("DBG_Q", [DK, S], F32, kind="ExternalOutput")
        dDBG_R = nc.dram_tensor("DBG_R", [1, S], F32, kind="ExternalOutput")
        dDBG_S = nc.dram_tensor("DBG_S", [1, S], F32, kind="ExternalOutput")
        dDBG_M = nc.dram_tensor("DBG_M", [128, S // 128], F32,
                                kind="ExternalOutput")

    with tile.TileContext(nc) as tc, ExitStack() as ctx:
        pp = ctx.enter_context(tc.tile_pool(name="pp", bufs=1))

        # ---- constants (whole-kernel scope) ------------------------------
        tri = pp.tile([128, 128], BF16, name="tri")
        onecf = pp.tile([128, 1], F16, name="onecf")
        onecb = pp.tile([128, 1], BF16, name="onecb")
        oner16 = pp.tile([1, 128], F16, name="oner16")
        onerR = pp.tile([1, 128], F32R, name="onerR")
        idt = pp.tile([128, 128], F16, name="idt")
        qb = pp.tile([128, NHL], F32, name="qb")
        shift_t = pp.tile([128, 1], F32, name="shift_t")
        eps_t = pp.tile([128, 1], F32, name="eps_t")
        nc.gpsimd.dma_start(out=tri, in_=dTRI[:, :])
        nc.gpsimd.dma_start(out=onecf, in_=dONEC[:, :])
        nc.gpsimd.dma_start(out=onecb, in_=dONEC[:, :])
        nc.gpsimd.dma_start(out=oner16, in_=dONER[:, :])
        nc.gpsimd.dma_start(out=onerR, in_=dONER[:, :])
        nc.gpsimd.dma_start(out=idt, in_=dIDT[:, :])
        nc.gpsimd.dma_start(out=qb, in_=dQB[:, :])
        nc.vector.memset(shift_t, -SHIFT)
        nc.vector.memset(eps_t, EPS)

        # persistent across phases: rope'd Q, V tiles, attention out, Wo w
        pmid = ctx.enter_context(tc.tile_pool(name="pmid", bufs=1))
        qr = [pmid.tile([DK, S], F16, name=f"qr{h}") for h in range(NHL)]
        vv = [pmid.tile([128, S], BF16, name=f"vv{h}") for h in range(NHL)]
        wo = pmid.tile([128, NHL, D], F16, name="wo")

        # =================== Phase A: load, RMS, Q proj ===================
        with tc.tile_pool(name="pxw", bufs=1) as pxw, \
             tc.tile_pool(name="psQ", bufs=3, space="PSUM") as psQ, \
             tc.tile_pool(name="psRow", bufs=4, space="PSUM") as psRow, \
             tc.tile_pool(name="psAux", bufs=1, space="PSUM") as psAux:

            cosa = pxw.tile([DK, S], F16, name="cosa")
            sina = pxw.tile([DK, S], F16, name="sina")
            xt = [pxw.tile([128, S], F16, name=f"xt{k}") for k in range(KT)]
            wq = pxw.tile([128, KT, ML], F16, name="wq")
            qth = [pxw.tile([DK, S], F16, name=f"qth{h}") for h in range(NHL)]

            nc.gpsimd.dma_start(out=cosa, in_=dCOSA[:, :])
            nc.gpsimd.dma_start(out=sina, in_=dSINA[:, :])
            for k in range(KT):
                nc.sync.dma_start(out=xt[k], in_=dXT[k * 128:(k + 1) * 128, :])
                nc.sync.dma_start(out=wq[:, k, :],
                                  in_=dWQT[k * 128:(k + 1) * 128, :])
            for h in range(NHL):
                nc.scalar.dma_start(out=wo[:, h, :],
                                    in_=dWOT[h * 128:(h + 1) * 128, :])

            # ssq rows: 4 x [1,512] PSUM rows (one bank each)
            ssq_ps = [psRow.tile([1, 512], F32, name=f"ssq{c}", tag="row")
                      for c in range(SK)]
            for k in range(KT):
                sq = pxw.tile([128, S], F16, name=f"sq{k}", tag="sq", bufs=2)
                nc.scalar.activation(sq, xt[k],
                                     mybir.ActivationFunctionType.Square)
                for c in range(SK):
                    nc.tensor.matmul(ssq_ps[c], onecf,
                                     sq[:, c * 512:(c + 1) * 512],
                                     start=(k == 0), stop=(k == KT - 1))

            # rms chain: rows -> [128,16] -> rsqrt -> back to rows -> Rbc
            sqrow = pxw.tile([1, S], F32, name="sqrow")
            for c in range(SK):
                nc.vector.tensor_copy(sqrow[0:1, c * 512:(c + 1) * 512],
                                      ssq_ps[c])
            onef32 = pxw.tile([128, 1], F32, name="onef32")
            nc.gpsimd.dma_start(out=onef32, in_=dONEC[:, :])
            idt32 = pxw.tile([128, 128], F32, name="idt32")
            nc.gpsimd.dma_start(out=idt32, in_=dIDT[:, :])
            auxa = psAux.tile([128, 512], F32, name="auxa", tag="aux")
            for c in range(SK):
                for j in range(4):
                    t = c * 4 + j
                    nc.tensor.transpose(
                        auxa[:, t:t + 1],
                        sqrow[0:1, c * 512 + j * 128:c * 512 + (j + 1) * 128],
                        onef32[0:1, 0:1])
            ms128 = pxw.tile([128, ST], F32, name="ms128")
            nc.vector.tensor_copy(ms128, auxa[:, 0:ST])
            # r = 1/sqrt(ms/D + eps) = exp(-0.5 * ln(ms/D + eps))
            lnms = pxw.tile([128, ST], F32, name="lnms")
            nc.scalar.activation(lnms, ms128,
                                 mybir.ActivationFunctionType.Ln,
                                 bias=eps_t, scale=1.0 / D)
            r128 = pxw.tile([128, ST], F16, name="r128")
            nc.scalar.activation(r128, lnms,
                                 mybir.ActivationFunctionType.Exp,
                                 scale=-0.5)
            # transpose back column-by-column into a partition-0 row
            rrow = pxw.tile([1, S], F16, name="rrow")
            auxb = psAux.tile([128, 1024], F16, name="auxb", tag="aux")
            for c in range(SK):
                for j in range(4):
                    t = c * 4 + j
                    nc.tensor.transpose(
                        auxb[0:1, j * 128:(j + 1) * 128],
                        r128[:, t:t + 1], idt)
                nc.vector.tensor_copy(rrow[0:1, c * 512:(c + 1) * 512],
                                      auxb[0:1, 0:512])

            # Rbc: broadcast r across partitions, [128, S] f16 in SBUF
            Rbc = pxw.tile([128, S], F16, name="Rbc")
            for c in range(SK):
                rbc_ps = psRow.tile([128, 512], F32, name=f"rbc{c}", tag="row")
                for j in range(4):
                    sl = slice(c * 512 + j * 128, c * 512 + (j + 1) * 128)
                    nc.tensor.matmul(rbc_ps[:, j * 128:(j + 1) * 128], oner16,
                                     rrow[0:1, sl],
                                     start=True, stop=True)
                nc.vector.tensor_copy(Rbc[:, c * 512:(c + 1) * 512], rbc_ps)

            # Q projection + per-head rope + V transposes, head-major
            for mt in range(NHL):
                for c in range(SK):
                    q_ps = psQ.tile([128, 512], F32, name=f"qps{mt}_{c}",
                                    tag="q")
                    for k in range(KT):
                        nc.tensor.matmul(
                            q_ps,
                            wq[:, k, mt * 128:(mt + 1) * 128],
                            xt[k][:, c * 512:(c + 1) * 512],
                            start=(k == 0), stop=(k == KT - 1))
                    sl = slice(c * 512, (c + 1) * 512)
                    nc.vector.tensor_mul(qth[mt][:, sl], q_ps, Rbc[:, sl])
                    nc.vector.tensor_scalar_add(qth[mt][:, sl], qth[mt][:, sl],
                                                qb[:, mt:mt + 1])
                # rope for this head
                sh = pxw.tile([DK, S], F16, name=f"sh{mt}", tag="ropesh",
                              bufs=2)
                hw = DK // 2
                nc.vector.tensor_copy(sh[0:hw, :], qth[mt][hw:DK, :])
                nc.vector.tensor_copy(sh[hw:DK, :], qth[mt][0:hw, :])
                m1 = pxw.tile([DK, S], F16, name=f"m1_{mt}", tag="ropem1",
                              bufs=2)
                nc.vector.tensor_mul(m1, qth[mt], cosa)
                nc.vector.tensor_mul(sh, sh, sina)
                nc.vector.tensor_add(qr[mt], m1, sh)
                # V = qth.T per head (PE transpose via identity)
                for g in range(ST // 4):
                    vaux = psRow.tile([128, 512], F16, name=f"vx{mt}_{g}",
                                      tag="row")
                    for j in range(4):
                        st = g * 4 + j
                        nc.tensor.transpose(
                            vaux[:, j * 128:(j + 1) * 128],
                            qth[mt][:, st * 128:(st + 1) * 128], idt)
                    nc.vector.tensor_copy(vv[mt][:, g * 512:(g + 1) * 512],
                                          vaux)

            if debug:
                nc.gpsimd.dma_start(out=dDBG_Q[:, :], in_=qth[0][:, :])
                nc.gpsimd.dma_start(out=dDBG_R[:, :], in_=rrow[:, :])
                nc.gpsimd.dma_start(out=dDBG_S[:, :], in_=sqrow[:, :])
                nc.gpsimd.dma_start(out=dDBG_M[:, :], in_=ms128[:, :])

        # =================== Phase B: attention + Wo ======================
        with tc.tile_pool(name="patt", bufs=1) as pat, \
             tc.tile_pool(name="psS", bufs=2, space="PSUM") as psS, \
             tc.tile_pool(name="psO", bufs=2, space="PSUM") as psO, \
             tc.tile_pool(name="psDn", bufs=2, space="PSUM") as psDn, \
             tc.tile_pool(name="psB2", bufs=1, space="PSUM") as psB2, \
             tc.tile_pool(name="psW", bufs=1, space="PSUM") as psW:

            ot = [pat.tile([DK, S], F16, name=f"ot{h}") for h in range(NHL)]
            NC = D // 512

            # per-block state carried between pipeline stages
            def new_block(h, qc):
                npt = min(ST, qc * 4 + 4)
                return {
                    "h": h, "qc": qc, "npt": npt,
                    "atq": [pat.tile([128, 512], BF16, name=f"at{h}_{qc}_{p}",
                                     tag=f"atq{p}", bufs=2)
                            for p in range(npt)],
                    "dn_ps": psDn.tile([1, 512], F32, name=f"dn{h}_{qc}",
                                       tag="dn"),
                    "o_ps": None,
                }

            def emit_score(blk, pt):
                h, qc = blk["h"], blk["qc"]
                off = pt * 128 - qc * 512
                lo = max(0, off)
                n = 512 - lo
                s_ps = psS.tile([128, 512], F32, name=f"sps{h}_{qc}_{pt}",
                                tag="s")
                nc.tensor.matmul(
                    s_ps[:, 0:n],
                    qr[h][:, pt * 128:(pt + 1) * 128],
                    qr[h][:, qc * 512 + lo:(qc + 1) * 512],
                    start=True, stop=True)
                nc.scalar.activation(blk["atq"][pt][:, lo:512], s_ps[:, 0:n],
                                     mybir.ActivationFunctionType.Exp,
                                     bias=shift_t)
                if off >= 0:
                    # diagonal block: triangular causal mask (keep p <= q)
                    nc.vector.tensor_mul(blk["atq"][pt][:, lo:lo + 128],
                                         blk["atq"][pt][:, lo:lo + 128], tri)

            def emit_dn(blk, pt):
                off = pt * 128 - blk["qc"] * 512
                lo = max(0, off)
                nc.tensor.matmul(blk["dn_ps"][0:1, lo:512],
                                 onecb, blk["atq"][pt][:, lo:512],
                                 start=(pt == 0), stop=(pt == blk["npt"] - 1),
                                 skip_group_check=True)

            def emit_av(blk, pt):
                h, qc = blk["h"], blk["qc"]
                off = pt * 128 - qc * 512
                lo = max(0, off)
                if blk["o_ps"] is None:
                    blk["o_ps"] = psO.tile([128, 512], F32, name=f"o{h}_{qc}",
                                           tag="o")
                nc.tensor.matmul(blk["o_ps"][:, lo:512],
                                 vv[h][:, pt * 128:(pt + 1) * 128],
                                 blk["atq"][pt][:, lo:512],
                                 start=(pt == 0), stop=(pt == blk["npt"] - 1),
                                 skip_group_check=True)

            def emit_norm(blk):
                h, qc = blk["h"], blk["qc"]
                # 1/dn = exp(-ln(dn)) on the Act engine (PSUM read, same
                # table set as the attention exp)
                lnrow = pat.tile([1, 512], F32, name=f"lnr{h}_{qc}",
                                 tag="lnrow", bufs=2)
                nc.scalar.activation(lnrow, blk["dn_ps"],
                                     mybir.ActivationFunctionType.Ln)
                rdrow_r = pat.tile([1, 512], F32R, name=f"rdh{h}_{qc}",
                                   tag="rdrow16", bufs=2)
                nc.scalar.activation(rdrow_r, lnrow,
                                     mybir.ActivationFunctionType.Exp,
                                     scale=-1.0)
                bc2 = psB2.tile([128, 512], F32, name=f"bc2{h}_{qc}", tag="b2")
                nc.tensor.matmul(bc2, onerR, rdrow_r,
                                 start=True, stop=True)
                rdb = pat.tile([128, 512], F32, name=f"rdb{h}_{qc}",
                               tag="rdb", bufs=2)
                nc.vector.tensor_copy(rdb, bc2)
                sl = slice(qc * 512, (qc + 1) * 512)
                nc.vector.tensor_mul(ot[h][:, sl], blk["o_ps"], rdb)

            # Wo work queue: one st-group = 4 ncc PSUM groups + copies + DMA
            def emit_wo_group(st):
                out_sb = pat.tile([128, D], F16, name=f"osb{st}", tag="osb",
                                  bufs=2)
                for ncc in range(NC):
                    wo_ps = psW.tile([128, 512], F32, name=f"wops{st}_{ncc}",
                                     tag="w")
                    for h in range(NHL):
                        nc.tensor.matmul(
                            wo_ps,
                            ot[h][:, st * 128:(st + 1) * 128],
                            wo[:, h, ncc * 512:(ncc + 1) * 512],
                            start=(h == 0), stop=(h == NHL - 1))
                    nc.vector.tensor_copy(out_sb[:, ncc * 512:(ncc + 1) * 512],
                                          wo_ps)
                nc.sync.dma_start(out=dOUT[st * 128:(st + 1) * 128, :],
                                  in_=out_sb)

            prev = None
            wo_pending = []   # st indices ready to emit
            for qc in range(SK):
                for h in range(NHL):
                    cur = new_block(h, qc)
                    np_c = cur["npt"]
                    np_p = prev["npt"] if prev else 0
                    n = max(np_c, np_p)
                    for i in range(n):
                        if i < np_c:
                            emit_score(cur, i)
                        if prev and i < np_p:
                            emit_dn(prev, i)
                            emit_av(prev, i)
                        # sprinkle one Wo group every ~5 iterations
                        if wo_pending and i % 5 == 4:
                            emit_wo_group(wo_pending.pop(0))
                    if prev:
                        emit_norm(prev)
                        if prev["h"] == NHL - 1:
                            # all heads of prev's qc normalized -> Wo ready
                            wo_pending.extend(range(prev["qc"] * 4,
                                                    prev["qc"] * 4 + 4))
                    if wo_pending:
                        emit_wo_group(wo_pending.pop(0))
                    prev = cur

            # drain: last block + remaining Wo groups
            for i in range(prev["npt"]):
                emit_dn(prev, i)
                emit_av(prev, i)
            emit_norm(prev)
            wo_pending.extend(range(prev["qc"] * 4, prev["qc"] * 4 + 4))
            while wo_pending:
                emit_wo_group(wo_pending.pop(0))

    return nc


# ======================= host-side preparation ===========================

def host_prep(X, Wq_w, Wq_b, Wo_w, Wo_b, rms_w, n_cores=8, NHL=4):
    """Build per-core input maps. X: (B,S,D) fp32."""
    B, S, D = X.shape
    DK = 128
    c = DK ** -0.25
    inv = 1.0 / (ROPE_BASE ** (np.arange(0, DK, 2, dtype=np.float64) / DK))
    ang = np.arange(S, dtype=np.float64)[:, None] * inv[None, :]
    cos = np.concatenate([np.cos(ang), np.cos(ang)], -1)     # (S, DK)
    sin = np.concatenate([np.sin(ang), np.sin(ang)], -1)
    COSA = (cos.T * c).astype(np.float16)                    # (DK, S)
    SINT = (sin.T * c).astype(np.float32)
    SINA = np.concatenate([-SINT[:DK // 2], SINT[DK // 2:]], 0).astype(np.float16)
    TRI = np.triu(np.ones((128, 128), np.float32)).astype(np.float16)  # p <= q
    ONEC = np.ones((128, 1), np.float32)
    ONER = np.ones((1, 128), np.float32)
    IDT = np.eye(128, dtype=np.float16)

    Wq_eff = (Wq_w * rms_w[None, :]).astype(np.float32)       # (D, D) fold rms
    in_maps = []
    groups = n_cores // B                                     # head-groups per batch
    ML = NHL * DK
    for core in range(n_cores):
        b = core // groups
        hg = core % groups
        msl = slice(hg * ML, (hg + 1) * ML)
        XT = np.ascontiguousarray(X[b].T).astype(np.float16)          # (D, S)
        WQT = np.ascontiguousarray(Wq_eff[msl, :].T).astype(np.float16)   # (D, ML)
        WOT = np.ascontiguousarray(Wo_w[:, msl].T).astype(np.float16)     # (ML, D)
        QB = np.ascontiguousarray(
            Wq_b[msl].reshape(NHL, 128).T).astype(np.float32)             # (128, NHL)
        in_maps.append({
            "XT": XT, "WQT": WQT, "WOT": WOT, "QB": QB,
            "COSA": COSA, "SINA": SINA, "TRI": TRI,
            "ONEC": ONEC, "ONER": ONER, "IDT": IDT,
        })
    return in_maps


def host_reduce(X, Wo_b, results, n_cores=8):
    B, S, D = X.shape
    groups = n_cores // B
    out = np.empty((B, S, D), np.float32)
    for b in range(B):
        acc = X[b].astype(np.float32).copy()
        for hg in range(groups):
            acc += results[b * groups + hg]["OUTP"].astype(np.float32)
        acc += Wo_b[None, :]
        out[b] = acc
    return out


# ======================= public entry point ==============================

_CACHE = {}


def _get_nc():
    if "nc" not in _CACHE:
        nc = build_core(S=2048, D=2048, NHL=4, DK=128, SHIFT=10.0)
        legalize_sync_waits(nc, max_waits=1)
        _CACHE["nc"] = nc
    return _CACHE["nc"]


def kernel(X, Wq_w, Wq_b, Wo_w, Wo_b, rms_w):
    """Full-input MHA block: returns X + MHA(RMSNorm(X)) as np.float32.

    Shards batch(2) x head-groups(4) across 8 NeuronCores; each core
    produces a partial output (its 4 heads through Wo); the host sums the
    four partials per batch and adds bias + residual.
    """
    from concourse.bass_utils import run_bass_kernel_spmd

    X = np.asarray(X, np.float32)
    Wq_w = np.asarray(Wq_w, np.float32)
    Wq_b = np.asarray(Wq_b, np.float32)
    Wo_w = np.asarray(Wo_w, np.float32)
    Wo_b = np.asarray(Wo_b, np.float32)
    rms_w = np.asarray(rms_w, np.float32)

    nc = _get_nc()
    in_maps = host_prep(X, Wq_w, Wq_b, Wo_w, Wo_b, rms_w)
    res = run_bass_kernel_spmd(nc, in_maps, core_ids=list(range(8)))
    return host_reduce(X, Wo_b, res.results)


# revision 41
# speedup vs baseline: 1.0873x; 1.0873x over previous
"""MHA kernel builder for TRN2 (per-core SPMD program) + host prep.

Problem: out = X + MHA(RMSNorm(X)) where Q=K=V=(RMSNorm(X)@Wq.T+b), rope,
causal softmax, Wo projection. Sharding: batch(2) x head-groups(4) over 8
cores; each core computes a partial of out[b] (its 4 heads through Wo);
host sums partials + bias + residual.

v2: fp16 everywhere on the PE path, software-pipelined attention emission
(scores/exp of block i interleaved with dn/AV of block i-1 and Wo of the
previous q-chunk), row reciprocals via the fast custom-DVE op, fp16 output.
"""
import math
import itertools
import numpy as np
from contextlib import ExitStack

import concourse.bass as bass
import concourse.mybir as mybir
import concourse.tile as tile

F32 = mybir.dt.float32
F32R = mybir.dt.float32r
F16 = mybir.dt.float16
BF16 = mybir.dt.bfloat16

EPS = float(np.finfo(np.float32).eps)
ROPE_BASE = 10000.0

_ctr = itertools.count()


def legalize_sync_waits(nc, max_waits=1):
    """This walrus accepts at most one sync-wait per instruction; hoist
    excess waits onto same-engine NOPs inserted just before."""
    n_fixed = 0
    for f in nc.m.functions:
        for bb in f.blocks:
            insts = bb.instructions
            out = []
            dirty = False
            for inst in insts:
                si = getattr(inst, "sync_info", None)
                if si is not None and si.on_wait and len(si.on_wait) > max_waits:
                    waits = list(si.on_wait)
                    for w in waits[:-max_waits]:
                        nop = mybir.InstNoOp(
                            name=f"I-syncfix-{next(_ctr)}", engine=inst.engine
                        )
                        nop.sync_info = mybir.SyncInfo(on_wait=[w], on_update=[])
                        nc.register_instruction(nop, overwrite=True)
                        out.append(nop)
                    inst.sync_info = mybir.SyncInfo(
                        on_wait=waits[-max_waits:], on_update=list(si.on_update or [])
                    )
                    dirty = True
                    n_fixed += 1
                out.append(inst)
            if dirty:
                bb.instructions = out
    return n_fixed


def build_core(S=2048, D=2048, NHL=4, DK=128, SHIFT=10.0, debug=False):
    """Emit the per-core program. Returns nc. All cores run this same NEFF
    with different input data."""
    assert S % 512 == 0 and D % 128 == 0 and DK == 128
    SK = S // 512     # 512-wide seq chunks
    KT = D // 128     # contraction tiles for projections
    ST = S // 128     # 128-wide seq tiles
    ML = NHL * DK     # local model width (q columns this core owns)

    nc = bass.Bass("TRN2", num_devices=8)
    dXT = nc.dram_tensor("XT", [D, S], F16, kind="ExternalInput")
    dWQT = nc.dram_tensor("WQT", [D, ML], F16, kind="ExternalInput")
    dWOT = nc.dram_tensor("WOT", [ML, D], F16, kind="ExternalInput")
    dQB = nc.dram_tensor("QB", [128, NHL], F32, kind="ExternalInput")
    dCOSA = nc.dram_tensor("COSA", [DK, S], F16, kind="ExternalInput")
    dSINA = nc.dram_tensor("SINA", [DK, S], F16, kind="ExternalInput")
    dTRI = nc.dram_tensor("TRI", [128, 128], F16, kind="ExternalInput")
    dONEC = nc.dram_tensor("ONEC", [128, 1], F32, kind="ExternalInput")
    dONER = nc.dram_tensor("ONER", [1, 128], F32, kind="ExternalInput")
    dIDT = nc.dram_tensor("IDT", [128, 128], F16, kind="ExternalInput")
    dOUT = nc.dram_tensor("OUTP", [S, D], F16, kind="ExternalOutput")
    if debug:
        dDBG_Q = nc.dram_tensor("DBG_Q", [DK, S], F32, kind="ExternalOutput")
        dDBG_R = nc.dram_tensor("DBG_R", [1, S], F32, kind="ExternalOutput")
        dDBG_S = nc.dram_tensor("DBG_S", [1, S], F32, kind="ExternalOutput")
        dDBG_M = nc.dram_tensor("DBG_M", [128, S // 128], F32,
                                kind="ExternalOutput")

    with tile.TileContext(nc) as tc, ExitStack() as ctx:
        pp = ctx.enter_context(tc.tile_pool(name="pp", bufs=1))

        # ---- constants (whole-kernel scope) ------------------------------
        tri = pp.tile([128, 128], BF16, name="tri")
        onecf = pp.tile([128, 1], F16, name="onecf")
        onecb = pp.tile([128, 1], BF16, name="onecb")
        oner16 = pp.tile([1, 128], F16, name="oner16")
        onerR = pp.tile([1, 128], F32R, name="onerR")
        idt = pp.tile([128, 128], F16, name="idt")
        qb = pp.tile([128, NHL], F32, name="qb")
        shift_t = pp.tile([128, 1], F32, name="shift_t")
        eps_t = pp.tile([128, 1], F32, name="eps_t")
        nc.gpsimd.dma_start(out=tri, in_=dTRI[:, :])
        nc.gpsimd.dma_start(out=onecf, in_=dONEC[:, :])
        nc.gpsimd.dma_start(out=onecb, in_=dONEC[:, :])
        nc.gpsimd.dma_start(out=oner16, in_=dONER[:, :])
        nc.gpsimd.dma_start(out=onerR, in_=dONER[:, :])
        nc.gpsimd.dma_start(out=idt, in_=dIDT[:, :])
        nc.gpsimd.dma_start(out=qb, in_=dQB[:, :])
        nc.vector.memset(shift_t, -SHIFT)
        nc.vector.memset(eps_t, EPS)

        # persistent across phases: rope'd Q, V tiles, attention out, Wo w
        pmid = ctx.enter_context(tc.tile_pool(name="pmid", bufs=1))
        qr = [pmid.tile([DK, S], F16, name=f"qr{h}") for h in range(NHL)]
        vv = [pmid.tile([128, S], BF16, name=f"vv{h}") for h in range(NHL)]
        wo = pmid.tile([128, NHL, D], F16, name="wo")

        # =================== Phase A: load, RMS, Q proj ===================
        with tc.tile_pool(name="pxw", bufs=1) as pxw, \
             tc.tile_pool(name="psQ", bufs=3, space="PSUM") as psQ, \
             tc.tile_pool(name="psRow", bufs=4, space="PSUM") as psRow, \
             tc.tile_pool(name="psAux", bufs=1, space="PSUM") as psAux:

            cosa = pxw.tile([DK, S], F16, name="cosa")
            sina = pxw.tile([DK, S], F16, name="sina")
            xt = [pxw.tile([128, S], F16, name=f"xt{k}") for k in range(KT)]
            wq = pxw.tile([128, KT, ML], F16, name="wq")
            qth = [pxw.tile([DK, S], F16, name=f"qth{h}") for h in range(NHL)]

            for k in range(KT):
                nc.sync.dma_start(out=xt[k], in_=dXT[k * 128:(k + 1) * 128, :])
                nc.sync.dma_start(out=wq[:, k, :],
                                  in_=dWQT[k * 128:(k + 1) * 128, :])

            # Q-proj combos; first wave overlaps the ssq/DMA window
            combos = [(mt, c) for mt in range(NHL) for c in range(SK)]
            wave0, rest = combos[:3], combos[3:]
            q_ps = {}
            for mt, c in wave0:
                q_ps[(mt, c)] = psQ.tile([128, 512], F32,
                                         name=f"qps{mt}_{c}", tag="q")

            # ssq rows: 4 x [1,512] PSUM rows (one bank each)
            ssq_ps = [psRow.tile([1, 512], F32, name=f"ssq{c}", tag="row")
                      for c in range(SK)]
            for k in range(KT):
                sq = pxw.tile([128, S], F16, name=f"sq{k}", tag="sq", bufs=2)
                nc.vector.tensor_mul(sq, xt[k], xt[k])
                for mt, c in wave0:
                    nc.tensor.matmul(
                        q_ps[(mt, c)],
                        wq[:, k, mt * 128:(mt + 1) * 128],
                        xt[k][:, c * 512:(c + 1) * 512],
                        start=(k == 0), stop=(k == KT - 1))
                for c in range(SK):
                    nc.tensor.matmul(ssq_ps[c], onecf,
                                     sq[:, c * 512:(c + 1) * 512],
                                     start=(k == 0), stop=(k == KT - 1))
            # bulk loads needed later: emit after the latency-critical xt
            nc.gpsimd.dma_start(out=cosa, in_=dCOSA[:, :])
            nc.gpsimd.dma_start(out=sina, in_=dSINA[:, :])
            for h in range(NHL):
                nc.scalar.dma_start(out=wo[:, h, :],
                                    in_=dWOT[h * 128:(h + 1) * 128, :])

            # rms chain: rows -> [128,16] -> rsqrt -> back to rows -> Rbc
            sqrow = pxw.tile([1, S], F32, name="sqrow")
            for c in range(SK):
                nc.vector.tensor_copy(sqrow[0:1, c * 512:(c + 1) * 512],
                                      ssq_ps[c])
            onef32 = pxw.tile([128, 1], F32, name="onef32")
            nc.gpsimd.dma_start(out=onef32, in_=dONEC[:, :])
            auxa = psAux.tile([128, 512], F32, name="auxa", tag="aux")
            for c in range(SK):
                for j in range(4):
                    t = c * 4 + j
                    nc.tensor.transpose(
                        auxa[:, t:t + 1],
                        sqrow[0:1, c * 512 + j * 128:c * 512 + (j + 1) * 128],
                        onef32[0:1, 0:1])
            ms128 = pxw.tile([128, ST], F32, name="ms128")
            nc.vector.tensor_copy(ms128, auxa[:, 0:ST])
            # r = 1/sqrt(ms/D + eps) = exp(-0.5 * ln(ms/D + eps))
            lnms = pxw.tile([128, ST], F32, name="lnms")
            nc.scalar.activation(lnms, ms128,
                                 mybir.ActivationFunctionType.Ln,
                                 bias=eps_t, scale=1.0 / D)
            r128 = pxw.tile([128, ST], F16, name="r128")
            nc.scalar.activation(r128, lnms,
                                 mybir.ActivationFunctionType.Exp,
                                 scale=-0.5)
            # transpose back column-by-column into a partition-0 row
            rrow = pxw.tile([1, S], F16, name="rrow")
            auxb = psAux.tile([128, 1024], F16, name="auxb", tag="aux")
            for c in range(SK):
                for j in range(4):
                    t = c * 4 + j
                    nc.tensor.transpose(
                        auxb[0:1, j * 128:(j + 1) * 128],
                        r128[:, t:t + 1], idt)
                nc.vector.tensor_copy(rrow[0:1, c * 512:(c + 1) * 512],
                                      auxb[0:1, 0:512])

            # Rbc: broadcast r across partitions, [128, S] f16 in SBUF
            Rbc = pxw.tile([128, S], F16, name="Rbc")
            for c in range(SK):
                rbc_ps = psRow.tile([128, 512], F32, name=f"rbc{c}", tag="row")
                for j in range(4):
                    sl = slice(c * 512 + j * 128, c * 512 + (j + 1) * 128)
                    nc.tensor.matmul(rbc_ps[:, j * 128:(j + 1) * 128], oner16,
                                     rrow[0:1, sl],
                                     start=True, stop=True)
                nc.vector.tensor_copy(Rbc[:, c * 512:(c + 1) * 512], rbc_ps)

            # Q projection drains + remaining combos; rope + V per done head
            def drain_combo(mt, c):
                sl = slice(c * 512, (c + 1) * 512)
                nc.vector.tensor_mul(qth[mt][:, sl], q_ps[(mt, c)], Rbc[:, sl])
                nc.vector.tensor_scalar_add(qth[mt][:, sl], qth[mt][:, sl],
                                            qb[:, mt:mt + 1])

            def finish_head(mt):
                # rope for this head
                sh = pxw.tile([DK, S], F16, name=f"sh{mt}", tag="ropesh",
                              bufs=2)
                hw = DK // 2
                nc.vector.tensor_copy(sh[0:hw, :], qth[mt][hw:DK, :])
                nc.vector.tensor_copy(sh[hw:DK, :], qth[mt][0:hw, :])
                m1 = pxw.tile([DK, S], F16, name=f"m1_{mt}", tag="ropem1",
                              bufs=2)
                nc.vector.tensor_mul(m1, qth[mt], cosa)
                nc.vector.tensor_mul(sh, sh, sina)
                nc.vector.tensor_add(qr[mt], m1, sh)
                # V = qth.T per head (PE transpose via identity)
                for g in range(ST // 4):
                    vaux = psRow.tile([128, 512], F16, name=f"vx{mt}_{g}",
                                      tag="row")
                    for j in range(4):
                        st = g * 4 + j
                        nc.tensor.transpose(
                            vaux[:, j * 128:(j + 1) * 128],
                            qth[mt][:, st * 128:(st + 1) * 128], idt)
                    nc.vector.tensor_copy(vv[mt][:, g * 512:(g + 1) * 512],
                                          vaux)

            for mt, c in wave0:
                drain_combo(mt, c)
            for mt, c in rest:
                q_ps[(mt, c)] = psQ.tile([128, 512], F32,
                                         name=f"qps{mt}_{c}", tag="q")
                for k in range(KT):
                    nc.tensor.matmul(
                        q_ps[(mt, c)],
                        wq[:, k, mt * 128:(mt + 1) * 128],
                        xt[k][:, c * 512:(c + 1) * 512],
                        start=(k == 0), stop=(k == KT - 1))
                drain_combo(mt, c)
                if c == SK - 1:
                    finish_head(mt)

            if debug:
                nc.gpsimd.dma_start(out=dDBG_Q[:, :], in_=qth[0][:, :])
                nc.gpsimd.dma_start(out=dDBG_R[:, :], in_=rrow[:, :])
                nc.gpsimd.dma_start(out=dDBG_S[:, :], in_=sqrow[:, :])
                nc.gpsimd.dma_start(out=dDBG_M[:, :], in_=ms128[:, :])

        # =================== Phase B: attention + Wo ======================
        with tc.tile_pool(name="patt", bufs=1) as pat, \
             tc.tile_pool(name="psS", bufs=2, space="PSUM") as psS, \
             tc.tile_pool(name="psO", bufs=2, space="PSUM") as psO, \
             tc.tile_pool(name="psDn", bufs=2, space="PSUM") as psDn, \
             tc.tile_pool(name="psB2", bufs=1, space="PSUM") as psB2, \
             tc.tile_pool(name="psW", bufs=1, space="PSUM") as psW:

            ot = [pat.tile([DK, S], F16, name=f"ot{h}") for h in range(NHL)]
            NC = D // 512

            # per-block state carried between pipeline stages
            def new_block(h, qc):
                npt = min(ST, qc * 4 + 4)
                return {
                    "h": h, "qc": qc, "npt": npt,
                    "atq": [pat.tile([128, 512], BF16, name=f"at{h}_{qc}_{p}",
                                     tag=f"atq{p}", bufs=2)
                            for p in range(npt)],
                    "dn_ps": psDn.tile([1, 512], F32, name=f"dn{h}_{qc}",
                                       tag="dn"),
                    "o_ps": None,
                }

            def emit_score(blk, pt):
                h, qc = blk["h"], blk["qc"]
                off = pt * 128 - qc * 512
                lo = max(0, off)
                n = 512 - lo
                s_ps = psS.tile([128, 512], F32, name=f"sps{h}_{qc}_{pt}",
                                tag="s")
                nc.tensor.matmul(
                    s_ps[:, 0:n],
                    qr[h][:, pt * 128:(pt + 1) * 128],
                    qr[h][:, qc * 512 + lo:(qc + 1) * 512],
                    start=True, stop=True)
                nc.scalar.activation(blk["atq"][pt][:, lo:512], s_ps[:, 0:n],
                                     mybir.ActivationFunctionType.Exp,
                                     bias=shift_t)
                if off >= 0:
                    # diagonal block: triangular causal mask (keep p <= q)
                    nc.vector.tensor_mul(blk["atq"][pt][:, lo:lo + 128],
                                         blk["atq"][pt][:, lo:lo + 128], tri)

            def emit_dn(blk, pt):
                off = pt * 128 - blk["qc"] * 512
                lo = max(0, off)
                nc.tensor.matmul(blk["dn_ps"][0:1, lo:512],
                                 onecb, blk["atq"][pt][:, lo:512],
                                 start=(pt == 0), stop=(pt == blk["npt"] - 1),
                                 skip_group_check=True)

            def emit_av(blk, pt):
                h, qc = blk["h"], blk["qc"]
                off = pt * 128 - qc * 512
                lo = max(0, off)
                if blk["o_ps"] is None:
                    blk["o_ps"] = psO.tile([128, 512], F32, name=f"o{h}_{qc}",
                                           tag="o")
                nc.tensor.matmul(blk["o_ps"][:, lo:512],
                                 vv[h][:, pt * 128:(pt + 1) * 128],
                                 blk["atq"][pt][:, lo:512],
                                 start=(pt == 0), stop=(pt == blk["npt"] - 1),
                                 skip_group_check=True)

            def emit_recip(blk):
                h, qc = blk["h"], blk["qc"]
                # 1/dn = exp(-ln(dn)) on the Act engine (PSUM read, same
                # table set as the attention exp)
                lnrow = pat.tile([1, 512], F32, name=f"lnr{h}_{qc}",
                                 tag="lnrow", bufs=2)
                nc.scalar.activation(lnrow, blk["dn_ps"],
                                     mybir.ActivationFunctionType.Ln)
                rdrow_r = pat.tile([1, 512], F32R, name=f"rdh{h}_{qc}",
                                   tag="rdrow16", bufs=2)
                nc.scalar.activation(rdrow_r, lnrow,
                                     mybir.ActivationFunctionType.Exp,
                                     scale=-1.0)
                bc2 = psB2.tile([128, 512], F32, name=f"bc2{h}_{qc}", tag="b2")
                nc.tensor.matmul(bc2, onerR, rdrow_r,
                                 start=True, stop=True)
                blk["bc2"] = bc2

            def emit_norm2(blk):
                h, qc = blk["h"], blk["qc"]
                rdb = pat.tile([128, 512], F32, name=f"rdb{h}_{qc}",
                               tag="rdb", bufs=2)
                nc.vector.tensor_copy(rdb, blk["bc2"])
                sl = slice(qc * 512, (qc + 1) * 512)
                nc.vector.tensor_mul(ot[h][:, sl], blk["o_ps"], rdb)

            # Wo work queue: one st-group = 4 ncc PSUM groups + copies + DMA
            def emit_wo_group(st, pools=None):
                out_sb = pat.tile([128, D], F16, name=f"osb{st}", tag="osb",
                                  bufs=2)
                pools = pools or [psW]
                for ncc in range(NC):
                    pw = pools[ncc % len(pools)]
                    wo_ps = pw.tile([128, 512], F32, name=f"wops{st}_{ncc}",
                                    tag="w" if pw is psW else "b2")
                    for h in range(NHL):
                        nc.tensor.matmul(
                            wo_ps,
                            ot[h][:, st * 128:(st + 1) * 128],
                            wo[:, h, ncc * 512:(ncc + 1) * 512],
                            start=(h == 0), stop=(h == NHL - 1))
                    nc.vector.tensor_copy(out_sb[:, ncc * 512:(ncc + 1) * 512],
                                          wo_ps)
                nc.sync.dma_start(out=dOUT[st * 128:(st + 1) * 128, :],
                                  in_=out_sb)

            prev = None
            wo_pending = []   # st indices ready to emit
            for qc in range(SK):
                for h in range(NHL):
                    cur = new_block(h, qc)
                    np_c = cur["npt"]
                    np_p = prev["npt"] if prev else 0
                    n = max(np_c, np_p)
                    for i in range(n):
                        if i < np_c:
                            emit_score(cur, i)
                        if prev and i < np_p:
                            emit_dn(prev, i)
                            emit_av(prev, i)
                            if i == np_p - 1:
                                emit_recip(prev)
                        # sprinkle one Wo group every ~5 iterations
                        if wo_pending and i % 5 == 4:
                            emit_wo_group(wo_pending.pop(0))
                    if prev:
                        emit_norm2(prev)
                        if prev["h"] == NHL - 1:
                            # all heads of prev's qc normalized -> Wo ready
                            wo_pending.extend(range(prev["qc"] * 4,
                                                    prev["qc"] * 4 + 4))
                    if wo_pending:
                        emit_wo_group(wo_pending.pop(0))
                    prev = cur

            # drain: last block + remaining Wo groups
            for i in range(prev["npt"]):
                emit_dn(prev, i)
                emit_av(prev, i)
            emit_recip(prev)
            emit_norm2(prev)
            wo_pending.extend(range(prev["qc"] * 4, prev["qc"] * 4 + 4))
            while wo_pending:
                emit_wo_group(wo_pending.pop(0), pools=[psW, psB2])

    return nc


# ======================= host-side preparation ===========================

def host_prep(X, Wq_w, Wq_b, Wo_w, Wo_b, rms_w, n_cores=8, NHL=4):
    """Build per-core input maps. X: (B,S,D) fp32."""
    B, S, D = X.shape
    DK = 128
    c = DK ** -0.25
    inv = 1.0 / (ROPE_BASE ** (np.arange(0, DK, 2, dtype=np.float64) / DK))
    ang = np.arange(S, dtype=np.float64)[:, None] * inv[None, :]
    cos = np.concatenate([np.cos(ang), np.cos(ang)], -1)     # (S, DK)
    sin = np.concatenate([np.sin(ang), np.sin(ang)], -1)
    COSA = (cos.T * c).astype(np.float16)                    # (DK, S)
    SINT = (sin.T * c).astype(np.float32)
    SINA = np.concatenate([-SINT[:DK // 2], SINT[DK // 2:]], 0).astype(np.float16)
    TRI = np.triu(np.ones((128, 128), np.float32)).astype(np.float16)  # p <= q
    ONEC = np.ones((128, 1), np.float32)
    ONER = np.ones((1, 128), np.float32)
    IDT = np.eye(128, dtype=np.float16)

    Wq_eff = (Wq_w * rms_w[None, :]).astype(np.float32)       # (D, D) fold rms
    in_maps = []
    groups = n_cores // B                                     # head-groups per batch
    ML = NHL * DK
    for core in range(n_cores):
        b = core // groups
        hg = core % groups
        msl = slice(hg * ML, (hg + 1) * ML)
        XT = np.ascontiguousarray(X[b].T).astype(np.float16)          # (D, S)
        WQT = np.ascontiguousarray(Wq_eff[msl, :].T).astype(np.float16)   # (D, ML)
        WOT = np.ascontiguousarray(Wo_w[:, msl].T).astype(np.float16)     # (ML, D)
        QB = np.ascontiguousarray(
            Wq_b[msl].reshape(NHL, 128).T).astype(np.float32)             # (128, NHL)
        in_maps.append({
            "XT": XT, "WQT": WQT, "WOT": WOT, "QB": QB,
            "COSA": COSA, "SINA": SINA, "TRI": TRI,
            "ONEC": ONEC, "ONER": ONER, "IDT": IDT,
        })
    return in_maps


def host_reduce(X, Wo_b, results, n_cores=8):
    B, S, D = X.shape
    groups = n_cores // B
    out = np.empty((B, S, D), np.float32)
    for b in range(B):
        acc = X[b].astype(np.float32).copy()
        for hg in range(groups):
            acc += results[b * groups + hg]["OUTP"].astype(np.float32)
        acc += Wo_b[None, :]
        out[b] = acc
    return out


# ======================= public entry point ==============================

_CACHE = {}


def _get_nc():
    if "nc" not in _CACHE:
        nc = build_core(S=2048, D=2048, NHL=4, DK=128, SHIFT=10.0)
        legalize_sync_waits(nc, max_waits=1)
        _CACHE["nc"] = nc
    return _CACHE["nc"]


def kernel(X, Wq_w, Wq_b, Wo_w, Wo_b, rms_w):
    """Full-input MHA block: returns X + MHA(RMSNorm(X)) as np.float32.

    Shards batch(2) x head-groups(4) across 8 NeuronCores; each core
    produces a partial output (its 4 heads through Wo); the host sums the
    four partials per batch and adds bias + residual.
    """
    from concourse.bass_utils import run_bass_kernel_spmd

    X = np.asarray(X, np.float32)
    Wq_w = np.asarray(Wq_w, np.float32)
    Wq_b = np.asarray(Wq_b, np.float32)
    Wo_w = np.asarray(Wo_w, np.float32)
    Wo_b = np.asarray(Wo_b, np.float32)
    rms_w = np.asarray(rms_w, np.float32)

    nc = _get_nc()
    in_maps = host_prep(X, Wq_w, Wq_b, Wo_w, Wo_b, rms_w)
    res = run_bass_kernel_spmd(nc, in_maps, core_ids=list(range(8)))
    return host_reduce(X, Wo_b, res.results)


# revision 45
# speedup vs baseline: 1.1293x; 1.0386x over previous
"""MHA kernel builder for TRN2 (per-core SPMD program) + host prep.

Problem: out = X + MHA(RMSNorm(X)) where Q=K=V=(RMSNorm(X)@Wq.T+b), rope,
causal softmax, Wo projection. Sharding: batch(2) x head-groups(4) over 8
cores; each core computes a partial of out[b] (its 4 heads through Wo);
host sums partials + bias + residual.

v2: fp16 everywhere on the PE path, software-pipelined attention emission
(scores/exp of block i interleaved with dn/AV of block i-1 and Wo of the
previous q-chunk), row reciprocals via the fast custom-DVE op, fp16 output.
"""
import math
import itertools
import numpy as np
from contextlib import ExitStack

import concourse.bass as bass
import concourse.mybir as mybir
import concourse.tile as tile

F32 = mybir.dt.float32
F32R = mybir.dt.float32r
F16 = mybir.dt.float16
BF16 = mybir.dt.bfloat16

EPS = float(np.finfo(np.float32).eps)
ROPE_BASE = 10000.0

_ctr = itertools.count()


def legalize_sync_waits(nc, max_waits=1):
    """This walrus accepts at most one sync-wait per instruction; hoist
    excess waits onto same-engine NOPs inserted just before."""
    n_fixed = 0
    for f in nc.m.functions:
        for bb in f.blocks:
            insts = bb.instructions
            out = []
            dirty = False
            for inst in insts:
                si = getattr(inst, "sync_info", None)
                if si is not None and si.on_wait and len(si.on_wait) > max_waits:
                    waits = list(si.on_wait)
                    for w in waits[:-max_waits]:
                        nop = mybir.InstNoOp(
                            name=f"I-syncfix-{next(_ctr)}", engine=inst.engine
                        )
                        nop.sync_info = mybir.SyncInfo(on_wait=[w], on_update=[])
                        nc.register_instruction(nop, overwrite=True)
                        out.append(nop)
                    inst.sync_info = mybir.SyncInfo(
                        on_wait=waits[-max_waits:], on_update=list(si.on_update or [])
                    )
                    dirty = True
                    n_fixed += 1
                out.append(inst)
            if dirty:
                bb.instructions = out
    return n_fixed


def build_core(S=2048, D=2048, NHL=4, DK=128, SHIFT=10.0, debug=False):
    """Emit the per-core program. Returns nc. All cores run this same NEFF
    with different input data."""
    assert S % 512 == 0 and D % 128 == 0 and DK == 128
    SK = S // 512     # 512-wide seq chunks
    KT = D // 128     # contraction tiles for projections
    ST = S // 128     # 128-wide seq tiles
    ML = NHL * DK     # local model width (q columns this core owns)

    nc = bass.Bass("TRN2", num_devices=8)
    dXT = nc.dram_tensor("XT", [D, S], F16, kind="ExternalInput")
    dWQT = nc.dram_tensor("WQT", [D, ML], F16, kind="ExternalInput")
    dWOT = nc.dram_tensor("WOT", [ML, D], F16, kind="ExternalInput")
    dQB = nc.dram_tensor("QB", [128, NHL], F32, kind="ExternalInput")
    dCOSA = nc.dram_tensor("COSA", [DK, S], F16, kind="ExternalInput")
    dSINA = nc.dram_tensor("SINA", [DK, S], F16, kind="ExternalInput")
    dTRI = nc.dram_tensor("TRI", [128, 128], F16, kind="ExternalInput")
    dONEC = nc.dram_tensor("ONEC", [128, 1], F32, kind="ExternalInput")
    dONER = nc.dram_tensor("ONER", [1, 128], F32, kind="ExternalInput")
    dIDT = nc.dram_tensor("IDT", [128, 128], F16, kind="ExternalInput")
    dOUT = nc.dram_tensor("OUTP", [S, D], F16, kind="ExternalOutput")
    if debug:
        dDBG_Q = nc.dram_tensor("DBG_Q", [DK, S], F32, kind="ExternalOutput")
        dDBG_R = nc.dram_tensor("DBG_R", [1, S], F32, kind="ExternalOutput")
        dDBG_S = nc.dram_tensor("DBG_S", [1, S], F32, kind="ExternalOutput")
        dDBG_M = nc.dram_tensor("DBG_M", [128, S // 128], F32,
                                kind="ExternalOutput")

    with tile.TileContext(nc) as tc, ExitStack() as ctx:
        pp = ctx.enter_context(tc.tile_pool(name="pp", bufs=1))

        # ---- constants (whole-kernel scope) ------------------------------
        tri = pp.tile([128, 128], BF16, name="tri")
        onecf = pp.tile([128, 1], F16, name="onecf")
        onecb = pp.tile([128, 1], BF16, name="onecb")
        oner16 = pp.tile([1, 128], F16, name="oner16")
        onerR = pp.tile([1, 128], F32R, name="onerR")
        idt = pp.tile([128, 128], F16, name="idt")
        qb = pp.tile([128, NHL], F32, name="qb")
        shift_t = pp.tile([128, 1], F32, name="shift_t")
        eps_t = pp.tile([128, 1], F32, name="eps_t")
        nc.gpsimd.dma_start(out=tri, in_=dTRI[:, :])
        nc.gpsimd.dma_start(out=onecf, in_=dONEC[:, :])
        nc.gpsimd.dma_start(out=onecb, in_=dONEC[:, :])
        nc.gpsimd.dma_start(out=oner16, in_=dONER[:, :])
        nc.gpsimd.dma_start(out=onerR, in_=dONER[:, :])
        nc.gpsimd.dma_start(out=idt, in_=dIDT[:, :])
        nc.gpsimd.dma_start(out=qb, in_=dQB[:, :])
        nc.vector.memset(shift_t, -SHIFT)
        nc.vector.memset(eps_t, EPS)

        # persistent across phases: rope'd Q, V tiles, attention out, Wo w.
        # qth/cosa/sina are persistent too so the tail head's rope (DVE)
        # doesn't gate the phase-B pool allocations (SBUF reuse barrier).
        pmid = ctx.enter_context(tc.tile_pool(name="pmid", bufs=1))
        qr = [pmid.tile([DK, S], F16, name=f"qr{h}") for h in range(NHL)]
        vv = [pmid.tile([128, S], BF16, name=f"vv{h}") for h in range(NHL)]
        wo = pmid.tile([128, NHL, D], F16, name="wo")
        qth = [pmid.tile([DK, S], F16, name=f"qth{h}") for h in range(NHL)]
        cosa = pmid.tile([DK, S], F16, name="cosa")
        sina = pmid.tile([DK, S], F16, name="sina")

        # =================== Phase A: load, RMS, Q proj ===================
        with tc.tile_pool(name="pxw", bufs=1) as pxw, \
             tc.tile_pool(name="psQ", bufs=3, space="PSUM") as psQ, \
             tc.tile_pool(name="psRow", bufs=4, space="PSUM") as psRow, \
             tc.tile_pool(name="psAux", bufs=1, space="PSUM") as psAux:

            xt = [pxw.tile([128, S], F16, name=f"xt{k}") for k in range(KT)]
            wq = pxw.tile([128, KT, ML], F16, name="wq")

            for k in range(KT):
                nc.sync.dma_start(out=xt[k], in_=dXT[k * 128:(k + 1) * 128, :])
                nc.sync.dma_start(out=wq[:, k, :],
                                  in_=dWQT[k * 128:(k + 1) * 128, :])

            # Q-proj combos; first wave overlaps the ssq/DMA window
            combos = [(mt, c) for mt in range(NHL) for c in range(SK)]
            wave0, rest = combos[:3], combos[3:]
            q_ps = {}
            for mt, c in wave0:
                q_ps[(mt, c)] = psQ.tile([128, 512], F32,
                                         name=f"qps{mt}_{c}", tag="q")

            # ssq rows: 4 x [1,512] PSUM rows (one bank each)
            ssq_ps = [psRow.tile([1, 512], F32, name=f"ssq{c}", tag="row")
                      for c in range(SK)]
            for k in range(KT):
                sq = pxw.tile([128, S], F16, name=f"sq{k}", tag="sq", bufs=2)
                nc.vector.tensor_mul(sq, xt[k], xt[k])
                for mt, c in wave0:
                    nc.tensor.matmul(
                        q_ps[(mt, c)],
                        wq[:, k, mt * 128:(mt + 1) * 128],
                        xt[k][:, c * 512:(c + 1) * 512],
                        start=(k == 0), stop=(k == KT - 1))
                for c in range(SK):
                    nc.tensor.matmul(ssq_ps[c], onecf,
                                     sq[:, c * 512:(c + 1) * 512],
                                     start=(k == 0), stop=(k == KT - 1))
            # bulk loads needed later: same queue as xt/wq so their
            # transfers don't steal HBM bandwidth from the critical xt path
            nc.sync.dma_start(out=cosa, in_=dCOSA[:, :])
            nc.sync.dma_start(out=sina, in_=dSINA[:, :])
            for h in range(NHL):
                nc.sync.dma_start(out=wo[:, h, :],
                                  in_=dWOT[h * 128:(h + 1) * 128, :])

            # rms chain: rows -> [128,16] -> rsqrt -> back to rows -> Rbc
            sqrow = pxw.tile([1, S], F32, name="sqrow")
            for c in range(SK):
                nc.vector.tensor_copy(sqrow[0:1, c * 512:(c + 1) * 512],
                                      ssq_ps[c])
            onef32 = pxw.tile([128, 1], F32, name="onef32")
            nc.gpsimd.dma_start(out=onef32, in_=dONEC[:, :])
            auxa = psAux.tile([128, 512], F32, name="auxa", tag="aux")
            for c in range(SK):
                for j in range(4):
                    t = c * 4 + j
                    nc.tensor.transpose(
                        auxa[:, t:t + 1],
                        sqrow[0:1, c * 512 + j * 128:c * 512 + (j + 1) * 128],
                        onef32[0:1, 0:1])
            ms128 = pxw.tile([128, ST], F32, name="ms128")
            nc.vector.tensor_copy(ms128, auxa[:, 0:ST])
            # r = 1/sqrt(ms/D + eps) = exp(-0.5 * ln(ms/D + eps))
            lnms = pxw.tile([128, ST], F32, name="lnms")
            nc.scalar.activation(lnms, ms128,
                                 mybir.ActivationFunctionType.Ln,
                                 bias=eps_t, scale=1.0 / D)
            r128 = pxw.tile([128, ST], F16, name="r128")
            nc.scalar.activation(r128, lnms,
                                 mybir.ActivationFunctionType.Exp,
                                 scale=-0.5)
            # transpose back column-by-column into a partition-0 row
            rrow = pxw.tile([1, S], F16, name="rrow")
            auxb = psAux.tile([128, 1024], F16, name="auxb", tag="aux")
            for c in range(SK):
                for j in range(4):
                    t = c * 4 + j
                    nc.tensor.transpose(
                        auxb[0:1, j * 128:(j + 1) * 128],
                        r128[:, t:t + 1], idt)
                nc.vector.tensor_copy(rrow[0:1, c * 512:(c + 1) * 512],
                                      auxb[0:1, 0:512])

            # Rbc: broadcast r across partitions, [128, S] f16 in SBUF
            Rbc = pxw.tile([128, S], F16, name="Rbc")
            for c in range(SK):
                rbc_ps = psRow.tile([128, 512], F32, name=f"rbc{c}", tag="row")
                for j in range(4):
                    sl = slice(c * 512 + j * 128, c * 512 + (j + 1) * 128)
                    nc.tensor.matmul(rbc_ps[:, j * 128:(j + 1) * 128], oner16,
                                     rrow[0:1, sl],
                                     start=True, stop=True)
                nc.vector.tensor_copy(Rbc[:, c * 512:(c + 1) * 512], rbc_ps)

            # Q projection drains + remaining combos; rope + V per done head
            def drain_combo(mt, c):
                sl = slice(c * 512, (c + 1) * 512)
                nc.vector.tensor_mul(qth[mt][:, sl], q_ps[(mt, c)], Rbc[:, sl])
                nc.vector.tensor_scalar_add(qth[mt][:, sl], qth[mt][:, sl],
                                            qb[:, mt:mt + 1])

            def finish_head(mt):
                # V = qth.T per head (PE transpose via identity); first so
                # the psRow banks free before the rope runs on DVE
                for g in range(ST // 4):
                    vaux = psRow.tile([128, 512], F16, name=f"vx{mt}_{g}",
                                      tag="row")
                    for j in range(4):
                        st = g * 4 + j
                        nc.tensor.transpose(
                            vaux[:, j * 128:(j + 1) * 128],
                            qth[mt][:, st * 128:(st + 1) * 128], idt)
                    nc.vector.tensor_copy(vv[mt][:, g * 512:(g + 1) * 512],
                                          vaux)
                # rope for this head (reads persistent tiles only)
                sh = pmid.tile([DK, S], F16, name=f"sh{mt}", tag="ropesh",
                               bufs=2)
                hw = DK // 2
                nc.vector.tensor_copy(sh[0:hw, :], qth[mt][hw:DK, :])
                nc.vector.tensor_copy(sh[hw:DK, :], qth[mt][0:hw, :])
                m1 = pmid.tile([DK, S], F16, name=f"m1_{mt}", tag="ropem1",
                               bufs=2)
                nc.vector.tensor_mul(m1, qth[mt], cosa)
                nc.vector.tensor_mul(sh, sh, sina)
                nc.vector.tensor_add(qr[mt], m1, sh)

            for mt, c in wave0:
                drain_combo(mt, c)
            for mt, c in rest:
                q_ps[(mt, c)] = psQ.tile([128, 512], F32,
                                         name=f"qps{mt}_{c}", tag="q")
                for k in range(KT):
                    nc.tensor.matmul(
                        q_ps[(mt, c)],
                        wq[:, k, mt * 128:(mt + 1) * 128],
                        xt[k][:, c * 512:(c + 1) * 512],
                        start=(k == 0), stop=(k == KT - 1))
                drain_combo(mt, c)
                if c == SK - 1:
                    finish_head(mt)

            if debug:
                nc.gpsimd.dma_start(out=dDBG_Q[:, :], in_=qth[0][:, :])
                nc.gpsimd.dma_start(out=dDBG_R[:, :], in_=rrow[:, :])
                nc.gpsimd.dma_start(out=dDBG_S[:, :], in_=sqrow[:, :])
                nc.gpsimd.dma_start(out=dDBG_M[:, :], in_=ms128[:, :])

        # =================== Phase B: attention + Wo ======================
        with tc.tile_pool(name="patt", bufs=1) as pat, \
             tc.tile_pool(name="psS", bufs=2, space="PSUM") as psS, \
             tc.tile_pool(name="psO", bufs=2, space="PSUM") as psO, \
             tc.tile_pool(name="psDn", bufs=2, space="PSUM") as psDn, \
             tc.tile_pool(name="psB2", bufs=1, space="PSUM") as psB2, \
             tc.tile_pool(name="psW", bufs=1, space="PSUM") as psW:

            ot = [pat.tile([DK, S], F16, name=f"ot{h}") for h in range(NHL)]
            NC = D // 512

            # per-block state carried between pipeline stages
            def new_block(h, qc):
                npt = min(ST, qc * 4 + 4)
                return {
                    "h": h, "qc": qc, "npt": npt,
                    "atq": [pat.tile([128, 512], BF16, name=f"at{h}_{qc}_{p}",
                                     tag=f"atq{p}", bufs=2)
                            for p in range(npt)],
                    "dn_ps": psDn.tile([1, 512], F32, name=f"dn{h}_{qc}",
                                       tag="dn"),
                    "o_ps": None,
                }

            def emit_score(blk, pt):
                h, qc = blk["h"], blk["qc"]
                off = pt * 128 - qc * 512
                lo = max(0, off)
                n = 512 - lo
                s_ps = psS.tile([128, 512], F32, name=f"sps{h}_{qc}_{pt}",
                                tag="s")
                nc.tensor.matmul(
                    s_ps[:, 0:n],
                    qr[h][:, pt * 128:(pt + 1) * 128],
                    qr[h][:, qc * 512 + lo:(qc + 1) * 512],
                    start=True, stop=True)
                nc.scalar.activation(blk["atq"][pt][:, lo:512], s_ps[:, 0:n],
                                     mybir.ActivationFunctionType.Exp,
                                     bias=shift_t)
                if off >= 0:
                    # diagonal block: triangular causal mask (keep p <= q)
                    nc.vector.tensor_mul(blk["atq"][pt][:, lo:lo + 128],
                                         blk["atq"][pt][:, lo:lo + 128], tri)

            def emit_dn(blk, pt):
                off = pt * 128 - blk["qc"] * 512
                lo = max(0, off)
                nc.tensor.matmul(blk["dn_ps"][0:1, lo:512],
                                 onecb, blk["atq"][pt][:, lo:512],
                                 start=(pt == 0), stop=(pt == blk["npt"] - 1),
                                 skip_group_check=True)

            def emit_av(blk, pt):
                h, qc = blk["h"], blk["qc"]
                off = pt * 128 - qc * 512
                lo = max(0, off)
                if blk["o_ps"] is None:
                    blk["o_ps"] = psO.tile([128, 512], F32, name=f"o{h}_{qc}",
                                           tag="o")
                nc.tensor.matmul(blk["o_ps"][:, lo:512],
                                 vv[h][:, pt * 128:(pt + 1) * 128],
                                 blk["atq"][pt][:, lo:512],
                                 start=(pt == 0), stop=(pt == blk["npt"] - 1),
                                 skip_group_check=True)

            def emit_recip(blk):
                h, qc = blk["h"], blk["qc"]
                # 1/dn = exp(-ln(dn)) on the Act engine (PSUM read, same
                # table set as the attention exp)
                lnrow = pat.tile([1, 512], F32, name=f"lnr{h}_{qc}",
                                 tag="lnrow", bufs=2)
                nc.scalar.activation(lnrow, blk["dn_ps"],
                                     mybir.ActivationFunctionType.Ln)
                rdrow_r = pat.tile([1, 512], F32R, name=f"rdh{h}_{qc}",
                                   tag="rdrow16", bufs=2)
                nc.scalar.activation(rdrow_r, lnrow,
                                     mybir.ActivationFunctionType.Exp,
                                     scale=-1.0)
                bc2 = psB2.tile([128, 512], F32, name=f"bc2{h}_{qc}", tag="b2")
                nc.tensor.matmul(bc2, onerR, rdrow_r,
                                 start=True, stop=True)
                blk["bc2"] = bc2

            def emit_norm2(blk):
                h, qc = blk["h"], blk["qc"]
                rdb = pat.tile([128, 512], F32, name=f"rdb{h}_{qc}",
                               tag="rdb", bufs=2)
                nc.vector.tensor_copy(rdb, blk["bc2"])
                sl = slice(qc * 512, (qc + 1) * 512)
                nc.vector.tensor_mul(ot[h][:, sl], blk["o_ps"], rdb)

            # Wo work queue: one st-group = 4 ncc PSUM groups + copies + DMA
            def emit_wo_group(st, pools=None):
                out_sb = pat.tile([128, D], F16, name=f"osb{st}", tag="osb",
                                  bufs=2)
                pools = pools or [psW]
                for ncc in range(NC):
                    pw = pools[ncc % len(pools)]
                    wo_ps = pw.tile([128, 512], F32, name=f"wops{st}_{ncc}",
                                    tag="w" if pw is psW else "b2")
                    for h in range(NHL):
                        nc.tensor.matmul(
                            wo_ps,
                            ot[h][:, st * 128:(st + 1) * 128],
                            wo[:, h, ncc * 512:(ncc + 1) * 512],
                            start=(h == 0), stop=(h == NHL - 1))
                    nc.vector.tensor_copy(out_sb[:, ncc * 512:(ncc + 1) * 512],
                                          wo_ps)
                nc.sync.dma_start(out=dOUT[st * 128:(st + 1) * 128, :],
                                  in_=out_sb)

            prev = None
            wo_pending = []   # st indices ready to emit
            for qc in range(SK):
                for h in range(NHL):
                    cur = new_block(h, qc)
                    np_c = cur["npt"]
                    np_p = prev["npt"] if prev else 0
                    n = max(np_c, np_p)
                    for i in range(n):
                        if i < np_c:
                            emit_score(cur, i)
                        if prev and i < np_p:
                            emit_dn(prev, i)
                            emit_av(prev, i)
                            if i == np_p - 1:
                                emit_recip(prev)
                        # sprinkle one Wo group every ~5 iterations
                        if wo_pending and i % 5 == 4:
                            emit_wo_group(wo_pending.pop(0))
                    if prev:
                        emit_norm2(prev)
                        if prev["h"] == NHL - 1:
                            # all heads of prev's qc normalized -> Wo ready
                            wo_pending.extend(range(prev["qc"] * 4,
                                                    prev["qc"] * 4 + 4))
                    if wo_pending:
                        emit_wo_group(wo_pending.pop(0))
                    prev = cur

            # drain: last block + remaining Wo groups
            for i in range(prev["npt"]):
                emit_dn(prev, i)
                emit_av(prev, i)
            emit_recip(prev)
            emit_norm2(prev)
            wo_pending.extend(range(prev["qc"] * 4, prev["qc"] * 4 + 4))
            while wo_pending:
                emit_wo_group(wo_pending.pop(0), pools=[psW, psB2])

    return nc


# ======================= host-side preparation ===========================

def host_prep(X, Wq_w, Wq_b, Wo_w, Wo_b, rms_w, n_cores=8, NHL=4):
    """Build per-core input maps. X: (B,S,D) fp32."""
    B, S, D = X.shape
    DK = 128
    c = DK ** -0.25
    inv = 1.0 / (ROPE_BASE ** (np.arange(0, DK, 2, dtype=np.float64) / DK))
    ang = np.arange(S, dtype=np.float64)[:, None] * inv[None, :]
    cos = np.concatenate([np.cos(ang), np.cos(ang)], -1)     # (S, DK)
    sin = np.concatenate([np.sin(ang), np.sin(ang)], -1)
    COSA = (cos.T * c).astype(np.float16)                    # (DK, S)
    SINT = (sin.T * c).astype(np.float32)
    SINA = np.concatenate([-SINT[:DK // 2], SINT[DK // 2:]], 0).astype(np.float16)
    TRI = np.triu(np.ones((128, 128), np.float32)).astype(np.float16)  # p <= q
    ONEC = np.ones((128, 1), np.float32)
    ONER = np.ones((1, 128), np.float32)
    IDT = np.eye(128, dtype=np.float16)

    Wq_eff = (Wq_w * rms_w[None, :]).astype(np.float32)       # (D, D) fold rms
    in_maps = []
    groups = n_cores // B                                     # head-groups per batch
    ML = NHL * DK
    for core in range(n_cores):
        b = core // groups
        hg = core % groups
        msl = slice(hg * ML, (hg + 1) * ML)
        XT = np.ascontiguousarray(X[b].T).astype(np.float16)          # (D, S)
        WQT = np.ascontiguousarray(Wq_eff[msl, :].T).astype(np.float16)   # (D, ML)
        WOT = np.ascontiguousarray(Wo_w[:, msl].T).astype(np.float16)     # (ML, D)
        QB = np.ascontiguousarray(
            Wq_b[msl].reshape(NHL, 128).T).astype(np.float32)             # (128, NHL)
        in_maps.append({
            "XT": XT, "WQT": WQT, "WOT": WOT, "QB": QB,
            "COSA": COSA, "SINA": SINA, "TRI": TRI,
            "ONEC": ONEC, "ONER": ONER, "IDT": IDT,
        })
    return in_maps


def host_reduce(X, Wo_b, results, n_cores=8):
    B, S, D = X.shape
    groups = n_cores // B
    out = np.empty((B, S, D), np.float32)
    for b in range(B):
        acc = X[b].astype(np.float32).copy()
        for hg in range(groups):
            acc += results[b * groups + hg]["OUTP"].astype(np.float32)
        acc += Wo_b[None, :]
        out[b] = acc
    return out


# ======================= public entry point ==============================

_CACHE = {}


def _get_nc():
    if "nc" not in _CACHE:
        nc = build_core(S=2048, D=2048, NHL=4, DK=128, SHIFT=10.0)
        legalize_sync_waits(nc, max_waits=1)
        _CACHE["nc"] = nc
    return _CACHE["nc"]


def kernel(X, Wq_w, Wq_b, Wo_w, Wo_b, rms_w):
    """Full-input MHA block: returns X + MHA(RMSNorm(X)) as np.float32.

    Shards batch(2) x head-groups(4) across 8 NeuronCores; each core
    produces a partial output (its 4 heads through Wo); the host sums the
    four partials per batch and adds bias + residual.
    """
    from concourse.bass_utils import run_bass_kernel_spmd

    X = np.asarray(X, np.float32)
    Wq_w = np.asarray(Wq_w, np.float32)
    Wq_b = np.asarray(Wq_b, np.float32)
    Wo_w = np.asarray(Wo_w, np.float32)
    Wo_b = np.asarray(Wo_b, np.float32)
    rms_w = np.asarray(rms_w, np.float32)

    nc = _get_nc()
    in_maps = host_prep(X, Wq_w, Wq_b, Wo_w, Wo_b, rms_w)
    res = run_bass_kernel_spmd(nc, in_maps, core_ids=list(range(8)))
    return host_reduce(X, Wo_b, res.results)


# revision 48
# speedup vs baseline: 1.1495x; 1.0179x over previous
"""MHA kernel builder for TRN2 (per-core SPMD program) + host prep.

Problem: out = X + MHA(RMSNorm(X)) where Q=K=V=(RMSNorm(X)@Wq.T+b), rope,
causal softmax, Wo projection. Sharding: batch(2) x head-groups(4) over 8
cores; each core computes a partial of out[b] (its 4 heads through Wo);
host sums partials + bias + residual.

v2: fp16 everywhere on the PE path, software-pipelined attention emission
(scores/exp of block i interleaved with dn/AV of block i-1 and Wo of the
previous q-chunk), row reciprocals via the fast custom-DVE op, fp16 output.
"""
import math
import itertools
import numpy as np
from contextlib import ExitStack

import concourse.bass as bass
import concourse.mybir as mybir
import concourse.tile as tile

F32 = mybir.dt.float32
F32R = mybir.dt.float32r
F16 = mybir.dt.float16
BF16 = mybir.dt.bfloat16

EPS = float(np.finfo(np.float32).eps)
ROPE_BASE = 10000.0

_ctr = itertools.count()


def legalize_sync_waits(nc, max_waits=1):
    """This walrus accepts at most one sync-wait per instruction; hoist
    excess waits onto same-engine NOPs inserted just before."""
    n_fixed = 0
    for f in nc.m.functions:
        for bb in f.blocks:
            insts = bb.instructions
            out = []
            dirty = False
            for inst in insts:
                si = getattr(inst, "sync_info", None)
                if si is not None and si.on_wait and len(si.on_wait) > max_waits:
                    waits = list(si.on_wait)
                    for w in waits[:-max_waits]:
                        nop = mybir.InstNoOp(
                            name=f"I-syncfix-{next(_ctr)}", engine=inst.engine
                        )
                        nop.sync_info = mybir.SyncInfo(on_wait=[w], on_update=[])
                        nc.register_instruction(nop, overwrite=True)
                        out.append(nop)
                    inst.sync_info = mybir.SyncInfo(
                        on_wait=waits[-max_waits:], on_update=list(si.on_update or [])
                    )
                    dirty = True
                    n_fixed += 1
                out.append(inst)
            if dirty:
                bb.instructions = out
    return n_fixed


def build_core(S=2048, D=2048, NHL=4, DK=128, SHIFT=10.0, debug=False):
    """Emit the per-core program. Returns nc. All cores run this same NEFF
    with different input data."""
    assert S % 512 == 0 and D % 128 == 0 and DK == 128
    SK = S // 512     # 512-wide seq chunks
    KT = D // 128     # contraction tiles for projections
    ST = S // 128     # 128-wide seq tiles
    ML = NHL * DK     # local model width (q columns this core owns)

    nc = bass.Bass("TRN2", num_devices=8)
    dXT = nc.dram_tensor("XT", [D, S], F16, kind="ExternalInput")
    dWQT = nc.dram_tensor("WQT", [D, ML], F16, kind="ExternalInput")
    dWOT = nc.dram_tensor("WOT", [ML, D], F16, kind="ExternalInput")
    dQB = nc.dram_tensor("QB", [128, NHL], F32, kind="ExternalInput")
    dCOSA = nc.dram_tensor("COSA", [DK, S], F16, kind="ExternalInput")
    dSINA = nc.dram_tensor("SINA", [DK, S], F16, kind="ExternalInput")
    dTRI = nc.dram_tensor("TRI", [128, 128], F16, kind="ExternalInput")
    dONEC = nc.dram_tensor("ONEC", [128, 1], F32, kind="ExternalInput")
    dONER = nc.dram_tensor("ONER", [1, 128], F32, kind="ExternalInput")
    dIDT = nc.dram_tensor("IDT", [128, 128], F16, kind="ExternalInput")
    dOUT = nc.dram_tensor("OUTP", [S, D], F16, kind="ExternalOutput")
    if debug:
        dDBG_Q = nc.dram_tensor("DBG_Q", [DK, S], F32, kind="ExternalOutput")
        dDBG_R = nc.dram_tensor("DBG_R", [1, S], F32, kind="ExternalOutput")
        dDBG_S = nc.dram_tensor("DBG_S", [1, S], F32, kind="ExternalOutput")
        dDBG_M = nc.dram_tensor("DBG_M", [128, S // 128], F32,
                                kind="ExternalOutput")

    with tile.TileContext(nc) as tc, ExitStack() as ctx:
        pp = ctx.enter_context(tc.tile_pool(name="pp", bufs=1))

        # ---- constants (whole-kernel scope) ------------------------------
        tri = pp.tile([128, 128], BF16, name="tri")
        onecf = pp.tile([128, 1], F16, name="onecf")
        onecb = pp.tile([128, 1], BF16, name="onecb")
        oner16 = pp.tile([1, 128], F16, name="oner16")
        onerR = pp.tile([1, 128], F32R, name="onerR")
        idt = pp.tile([128, 128], F16, name="idt")
        qb = pp.tile([128, NHL], F32, name="qb")
        shift_t = pp.tile([128, 1], F32, name="shift_t")
        eps_t = pp.tile([128, 1], F32, name="eps_t")
        nc.gpsimd.dma_start(out=tri, in_=dTRI[:, :])
        nc.gpsimd.dma_start(out=onecf, in_=dONEC[:, :])
        nc.gpsimd.dma_start(out=onecb, in_=dONEC[:, :])
        nc.gpsimd.dma_start(out=oner16, in_=dONER[:, :])
        nc.gpsimd.dma_start(out=onerR, in_=dONER[:, :])
        nc.gpsimd.dma_start(out=idt, in_=dIDT[:, :])
        nc.gpsimd.dma_start(out=qb, in_=dQB[:, :])
        nc.vector.memset(shift_t, -SHIFT)
        nc.vector.memset(eps_t, EPS)

        # persistent across phases: rope'd Q, V tiles, attention out, Wo w.
        # qth/cosa/sina are persistent too so the tail head's rope (DVE)
        # doesn't gate the phase-B pool allocations (SBUF reuse barrier).
        pmid = ctx.enter_context(tc.tile_pool(name="pmid", bufs=1))
        qr = [pmid.tile([DK, S], F16, name=f"qr{h}") for h in range(NHL)]
        vv = [pmid.tile([128, S], BF16, name=f"vv{h}") for h in range(NHL)]
        wo = pmid.tile([128, NHL, D], F16, name="wo")
        qth = [pmid.tile([DK, S], F16, name=f"qth{h}") for h in range(NHL)]
        cosa = pmid.tile([DK, S], F16, name="cosa")
        sina = pmid.tile([DK, S], F16, name="sina")

        # =================== Phase A: load, RMS, Q proj ===================
        with tc.tile_pool(name="pxw", bufs=1) as pxw, \
             tc.tile_pool(name="psQ", bufs=3, space="PSUM") as psQ, \
             tc.tile_pool(name="psRow", bufs=4, space="PSUM") as psRow, \
             tc.tile_pool(name="psAux", bufs=1, space="PSUM") as psAux:

            xt = [pxw.tile([128, S], F16, name=f"xt{k}") for k in range(KT)]
            wq = pxw.tile([128, KT, ML], F16, name="wq")

            for k in range(KT):
                nc.sync.dma_start(out=xt[k], in_=dXT[k * 128:(k + 1) * 128, :])
                nc.sync.dma_start(out=wq[:, k, :],
                                  in_=dWQT[k * 128:(k + 1) * 128, :])

            # Q-proj combos; first wave overlaps the ssq/DMA window
            combos = [(mt, c) for mt in range(NHL) for c in range(SK)]
            wave0, rest = combos[:3], combos[3:]
            q_ps = {}
            for mt, c in wave0:
                q_ps[(mt, c)] = psQ.tile([128, 512], F32,
                                         name=f"qps{mt}_{c}", tag="q")

            # ssq rows: 4 x [1,512] PSUM rows (one bank each)
            ssq_ps = [psRow.tile([1, 512], F32, name=f"ssq{c}", tag="row")
                      for c in range(SK)]
            for k in range(KT):
                sq = pxw.tile([128, S], F16, name=f"sq{k}", tag="sq", bufs=2)
                nc.vector.tensor_mul(sq, xt[k], xt[k])
                for mt, c in wave0:
                    nc.tensor.matmul(
                        q_ps[(mt, c)],
                        wq[:, k, mt * 128:(mt + 1) * 128],
                        xt[k][:, c * 512:(c + 1) * 512],
                        start=(k == 0), stop=(k == KT - 1))
                for c in range(SK):
                    nc.tensor.matmul(ssq_ps[c], onecf,
                                     sq[:, c * 512:(c + 1) * 512],
                                     start=(k == 0), stop=(k == KT - 1))
            # bulk loads needed later: same queue as xt/wq so their
            # transfers don't steal HBM bandwidth from the critical xt path
            nc.sync.dma_start(out=cosa, in_=dCOSA[:, :])
            nc.sync.dma_start(out=sina, in_=dSINA[:, :])
            for h in range(NHL):
                nc.sync.dma_start(out=wo[:, h, :],
                                  in_=dWOT[h * 128:(h + 1) * 128, :])

            # rms chain: rows -> [128,16] -> rsqrt -> back to rows -> Rbc
            sqrow = pxw.tile([1, S], F32, name="sqrow")
            for c in range(SK):
                nc.vector.tensor_copy(sqrow[0:1, c * 512:(c + 1) * 512],
                                      ssq_ps[c])
            onef32 = pxw.tile([128, 1], F32, name="onef32")
            nc.gpsimd.dma_start(out=onef32, in_=dONEC[:, :])
            auxa = psAux.tile([128, 512], F32, name="auxa", tag="aux")
            for c in range(SK):
                for j in range(4):
                    t = c * 4 + j
                    nc.tensor.transpose(
                        auxa[:, t:t + 1],
                        sqrow[0:1, c * 512 + j * 128:c * 512 + (j + 1) * 128],
                        onef32[0:1, 0:1])
            ms128 = pxw.tile([128, ST], F32, name="ms128")
            nc.vector.tensor_copy(ms128, auxa[:, 0:ST])
            # r = 1/sqrt(ms/D + eps) = exp(-0.5 * ln(ms/D + eps))
            lnms = pxw.tile([128, ST], F32, name="lnms")
            nc.scalar.activation(lnms, ms128,
                                 mybir.ActivationFunctionType.Ln,
                                 bias=eps_t, scale=1.0 / D)
            r128 = pxw.tile([128, ST], F16, name="r128")
            nc.scalar.activation(r128, lnms,
                                 mybir.ActivationFunctionType.Exp,
                                 scale=-0.5)
            # transpose back column-by-column into a partition-0 row
            rrow = pxw.tile([1, S], F16, name="rrow")
            auxb = psAux.tile([128, 1024], F16, name="auxb", tag="aux")
            for c in range(SK):
                for j in range(4):
                    t = c * 4 + j
                    nc.tensor.transpose(
                        auxb[0:1, j * 128:(j + 1) * 128],
                        r128[:, t:t + 1], idt)
                nc.vector.tensor_copy(rrow[0:1, c * 512:(c + 1) * 512],
                                      auxb[0:1, 0:512])

            # Rbc: broadcast r across partitions, [128, S] f16 in SBUF
            Rbc = pxw.tile([128, S], F16, name="Rbc")
            for c in range(SK):
                rbc_ps = psRow.tile([128, 512], F32, name=f"rbc{c}", tag="row")
                for j in range(4):
                    sl = slice(c * 512 + j * 128, c * 512 + (j + 1) * 128)
                    nc.tensor.matmul(rbc_ps[:, j * 128:(j + 1) * 128], oner16,
                                     rrow[0:1, sl],
                                     start=True, stop=True)
                nc.vector.tensor_copy(Rbc[:, c * 512:(c + 1) * 512], rbc_ps)

            # Q projection drains + remaining combos; rope + V per done head
            def drain_combo(mt, c):
                sl = slice(c * 512, (c + 1) * 512)
                nc.vector.tensor_mul(qth[mt][:, sl], q_ps[(mt, c)], Rbc[:, sl])
                nc.vector.tensor_scalar_add(qth[mt][:, sl], qth[mt][:, sl],
                                            qb[:, mt:mt + 1])

            def finish_head(mt):
                # V = qth.T per head (PE transpose via identity); first so
                # the psRow banks free before the rope runs on DVE
                for g in range(ST // 4):
                    vaux = psRow.tile([128, 512], F16, name=f"vx{mt}_{g}",
                                      tag="row")
                    for j in range(4):
                        st = g * 4 + j
                        nc.tensor.transpose(
                            vaux[:, j * 128:(j + 1) * 128],
                            qth[mt][:, st * 128:(st + 1) * 128], idt)
                    nc.vector.tensor_copy(vv[mt][:, g * 512:(g + 1) * 512],
                                          vaux)
                # rope for this head (reads persistent tiles only)
                sh = pmid.tile([DK, S], F16, name=f"sh{mt}", tag="ropesh",
                               bufs=2)
                hw = DK // 2
                nc.vector.tensor_copy(sh[0:hw, :], qth[mt][hw:DK, :])
                nc.vector.tensor_copy(sh[hw:DK, :], qth[mt][0:hw, :])
                m1 = pmid.tile([DK, S], F16, name=f"m1_{mt}", tag="ropem1",
                               bufs=2)
                nc.vector.tensor_mul(m1, qth[mt], cosa)
                nc.vector.tensor_mul(sh, sh, sina)
                nc.vector.tensor_add(qr[mt], m1, sh)

            for mt, c in wave0:
                drain_combo(mt, c)
            for mt, c in rest:
                q_ps[(mt, c)] = psQ.tile([128, 512], F32,
                                         name=f"qps{mt}_{c}", tag="q")
                for k in range(KT):
                    nc.tensor.matmul(
                        q_ps[(mt, c)],
                        wq[:, k, mt * 128:(mt + 1) * 128],
                        xt[k][:, c * 512:(c + 1) * 512],
                        start=(k == 0), stop=(k == KT - 1))
                drain_combo(mt, c)
                if c == SK - 1:
                    finish_head(mt)

            if debug:
                nc.gpsimd.dma_start(out=dDBG_Q[:, :], in_=qth[0][:, :])
                nc.gpsimd.dma_start(out=dDBG_R[:, :], in_=rrow[:, :])
                nc.gpsimd.dma_start(out=dDBG_S[:, :], in_=sqrow[:, :])
                nc.gpsimd.dma_start(out=dDBG_M[:, :], in_=ms128[:, :])

        # =================== Phase B: attention + Wo ======================
        with tc.tile_pool(name="patt", bufs=1) as pat, \
             tc.tile_pool(name="psS", bufs=2, space="PSUM") as psS, \
             tc.tile_pool(name="psO", bufs=2, space="PSUM") as psO, \
             tc.tile_pool(name="psDn", bufs=2, space="PSUM") as psDn, \
             tc.tile_pool(name="psB2", bufs=1, space="PSUM") as psB2, \
             tc.tile_pool(name="psW", bufs=1, space="PSUM") as psW:

            ot = [pat.tile([DK, S], F16, name=f"ot{h}") for h in range(NHL)]
            NC = D // 512

            # per-block state carried between pipeline stages
            def new_block(h, qc):
                npt = min(ST, qc * 4 + 4)
                return {
                    "h": h, "qc": qc, "npt": npt,
                    "atq": [pat.tile([128, 512], BF16, name=f"at{h}_{qc}_{p}",
                                     tag=f"atq{p}", bufs=3)
                            for p in range(npt)],
                    "dn_ps": None,
                    "o_ps": None,
                }

            def emit_score(blk, pt):
                h, qc = blk["h"], blk["qc"]
                off = pt * 128 - qc * 512
                lo = max(0, off)
                n = 512 - lo
                s_ps = psS.tile([128, 512], F32, name=f"sps{h}_{qc}_{pt}",
                                tag="s")
                nc.tensor.matmul(
                    s_ps[:, 0:n],
                    qr[h][:, pt * 128:(pt + 1) * 128],
                    qr[h][:, qc * 512 + lo:(qc + 1) * 512],
                    start=True, stop=True)
                nc.scalar.activation(blk["atq"][pt][:, lo:512], s_ps[:, 0:n],
                                     mybir.ActivationFunctionType.Exp,
                                     bias=shift_t)
                if off >= 0:
                    # diagonal block: triangular causal mask (keep p <= q)
                    nc.vector.tensor_mul(blk["atq"][pt][:, lo:lo + 128],
                                         blk["atq"][pt][:, lo:lo + 128], tri)

            def emit_dn(blk, pt):
                off = pt * 128 - blk["qc"] * 512
                lo = max(0, off)
                if blk["dn_ps"] is None:
                    blk["dn_ps"] = psDn.tile([1, 512], F32,
                                             name=f"dn{blk['h']}_{blk['qc']}",
                                             tag="dn")
                nc.tensor.matmul(blk["dn_ps"][0:1, lo:512],
                                 onecb, blk["atq"][pt][:, lo:512],
                                 start=(pt == 0), stop=(pt == blk["npt"] - 1),
                                 skip_group_check=True)

            def emit_av(blk, pt):
                h, qc = blk["h"], blk["qc"]
                off = pt * 128 - qc * 512
                lo = max(0, off)
                if blk["o_ps"] is None:
                    blk["o_ps"] = psO.tile([128, 512], F32, name=f"o{h}_{qc}",
                                           tag="o")
                nc.tensor.matmul(blk["o_ps"][:, lo:512],
                                 vv[h][:, pt * 128:(pt + 1) * 128],
                                 blk["atq"][pt][:, lo:512],
                                 start=(pt == 0), stop=(pt == blk["npt"] - 1),
                                 skip_group_check=True)

            def emit_recip(blk):
                h, qc = blk["h"], blk["qc"]
                # 1/dn = exp(-ln(dn)) on the Act engine (PSUM read, same
                # table set as the attention exp)
                lnrow = pat.tile([1, 512], F32, name=f"lnr{h}_{qc}",
                                 tag="lnrow", bufs=2)
                nc.scalar.activation(lnrow, blk["dn_ps"],
                                     mybir.ActivationFunctionType.Ln)
                rdrow_r = pat.tile([1, 512], F32R, name=f"rdh{h}_{qc}",
                                   tag="rdrow16", bufs=2)
                nc.scalar.activation(rdrow_r, lnrow,
                                     mybir.ActivationFunctionType.Exp,
                                     scale=-1.0)
                bc2 = psB2.tile([128, 512], F32, name=f"bc2{h}_{qc}", tag="b2")
                nc.tensor.matmul(bc2, onerR, rdrow_r,
                                 start=True, stop=True)
                blk["bc2"] = bc2

            def emit_norm2(blk):
                h, qc = blk["h"], blk["qc"]
                rdb = pat.tile([128, 512], F32, name=f"rdb{h}_{qc}",
                               tag="rdb", bufs=2)
                nc.vector.tensor_copy(rdb, blk["bc2"])
                sl = slice(qc * 512, (qc + 1) * 512)
                nc.vector.tensor_mul(ot[h][:, sl], blk["o_ps"], rdb)

            # Wo work queue: one st-group = 4 ncc PSUM groups + copies + DMA
            def emit_wo_group(st, pools=None):
                out_sb = pat.tile([128, D], F16, name=f"osb{st}", tag="osb",
                                  bufs=2)
                pools = pools or [psW]
                for ncc in range(NC):
                    pw = pools[ncc % len(pools)]
                    wo_ps = pw.tile([128, 512], F32, name=f"wops{st}_{ncc}",
                                    tag="w" if pw is psW else "b2")
                    for h in range(NHL):
                        nc.tensor.matmul(
                            wo_ps,
                            ot[h][:, st * 128:(st + 1) * 128],
                            wo[:, h, ncc * 512:(ncc + 1) * 512],
                            start=(h == 0), stop=(h == NHL - 1))
                    nc.vector.tensor_copy(out_sb[:, ncc * 512:(ncc + 1) * 512],
                                          wo_ps)
                nc.sync.dma_start(out=dOUT[st * 128:(st + 1) * 128, :],
                                  in_=out_sb)

            # depth-2 software pipeline: scores/exp of block i interleaved
            # with dn/AV of block i-2, so PE never waits on the Act engine
            prev = prev2 = None
            wo_pending = []   # st indices ready to emit
            for qc in range(SK):
                for h in range(NHL):
                    cur = new_block(h, qc)
                    np_c = cur["npt"]
                    np_p = prev2["npt"] if prev2 else 0
                    n = max(np_c, np_p)
                    for i in range(n):
                        if i < np_c:
                            emit_score(cur, i)
                        if prev2 and i < np_p:
                            emit_dn(prev2, i)
                            emit_av(prev2, i)
                            if i == np_p - 1:
                                emit_recip(prev2)
                        # sprinkle one Wo group every ~5 iterations
                        if wo_pending and i % 5 == 4:
                            emit_wo_group(wo_pending.pop(0))
                    if prev2:
                        emit_norm2(prev2)
                        if prev2["h"] == NHL - 1:
                            # all heads of prev2's qc normalized -> Wo ready
                            wo_pending.extend(range(prev2["qc"] * 4,
                                                    prev2["qc"] * 4 + 4))
                    if wo_pending:
                        emit_wo_group(wo_pending.pop(0))
                    prev2, prev = prev, cur

            # drain the two in-flight blocks + remaining Wo groups
            for blk in (prev2, prev):
                for i in range(blk["npt"]):
                    emit_dn(blk, i)
                    emit_av(blk, i)
                    if wo_pending and i % 5 == 4:
                        emit_wo_group(wo_pending.pop(0))
                emit_recip(blk)
                emit_norm2(blk)
                if blk["h"] == NHL - 1:
                    wo_pending.extend(range(blk["qc"] * 4, blk["qc"] * 4 + 4))
            while wo_pending:
                emit_wo_group(wo_pending.pop(0), pools=[psW, psB2])

    return nc


# ======================= host-side preparation ===========================

def host_prep(X, Wq_w, Wq_b, Wo_w, Wo_b, rms_w, n_cores=8, NHL=4):
    """Build per-core input maps. X: (B,S,D) fp32."""
    B, S, D = X.shape
    DK = 128
    c = DK ** -0.25
    inv = 1.0 / (ROPE_BASE ** (np.arange(0, DK, 2, dtype=np.float64) / DK))
    ang = np.arange(S, dtype=np.float64)[:, None] * inv[None, :]
    cos = np.concatenate([np.cos(ang), np.cos(ang)], -1)     # (S, DK)
    sin = np.concatenate([np.sin(ang), np.sin(ang)], -1)
    COSA = (cos.T * c).astype(np.float16)                    # (DK, S)
    SINT = (sin.T * c).astype(np.float32)
    SINA = np.concatenate([-SINT[:DK // 2], SINT[DK // 2:]], 0).astype(np.float16)
    TRI = np.triu(np.ones((128, 128), np.float32)).astype(np.float16)  # p <= q
    ONEC = np.ones((128, 1), np.float32)
    ONER = np.ones((1, 128), np.float32)
    IDT = np.eye(128, dtype=np.float16)

    Wq_eff = (Wq_w * rms_w[None, :]).astype(np.float32)       # (D, D) fold rms
    in_maps = []
    groups = n_cores // B                                     # head-groups per batch
    ML = NHL * DK
    for core in range(n_cores):
        b = core // groups
        hg = core % groups
        msl = slice(hg * ML, (hg + 1) * ML)
        XT = np.ascontiguousarray(X[b].T).astype(np.float16)          # (D, S)
        WQT = np.ascontiguousarray(Wq_eff[msl, :].T).astype(np.float16)   # (D, ML)
        WOT = np.ascontiguousarray(Wo_w[:, msl].T).astype(np.float16)     # (ML, D)
        QB = np.ascontiguousarray(
            Wq_b[msl].reshape(NHL, 128).T).astype(np.float32)             # (128, NHL)
        in_maps.append({
            "XT": XT, "WQT": WQT, "WOT": WOT, "QB": QB,
            "COSA": COSA, "SINA": SINA, "TRI": TRI,
            "ONEC": ONEC, "ONER": ONER, "IDT": IDT,
        })
    return in_maps


def host_reduce(X, Wo_b, results, n_cores=8):
    B, S, D = X.shape
    groups = n_cores // B
    out = np.empty((B, S, D), np.float32)
    for b in range(B):
        acc = X[b].astype(np.float32).copy()
        for hg in range(groups):
            acc += results[b * groups + hg]["OUTP"].astype(np.float32)
        acc += Wo_b[None, :]
        out[b] = acc
    return out


# ======================= public entry point ==============================

_CACHE = {}


def _get_nc():
    if "nc" not in _CACHE:
        nc = build_core(S=2048, D=2048, NHL=4, DK=128, SHIFT=10.0)
        legalize_sync_waits(nc, max_waits=1)
        _CACHE["nc"] = nc
    return _CACHE["nc"]


def kernel(X, Wq_w, Wq_b, Wo_w, Wo_b, rms_w):
    """Full-input MHA block: returns X + MHA(RMSNorm(X)) as np.float32.

    Shards batch(2) x head-groups(4) across 8 NeuronCores; each core
    produces a partial output (its 4 heads through Wo); the host sums the
    four partials per batch and adds bias + residual.
    """
    from concourse.bass_utils import run_bass_kernel_spmd

    X = np.asarray(X, np.float32)
    Wq_w = np.asarray(Wq_w, np.float32)
    Wq_b = np.asarray(Wq_b, np.float32)
    Wo_w = np.asarray(Wo_w, np.float32)
    Wo_b = np.asarray(Wo_b, np.float32)
    rms_w = np.asarray(rms_w, np.float32)

    nc = _get_nc()
    in_maps = host_prep(X, Wq_w, Wq_b, Wo_w, Wo_b, rms_w)
    res = run_bass_kernel_spmd(nc, in_maps, core_ids=list(range(8)))
    return host_reduce(X, Wo_b, res.results)


# revision 55
# speedup vs baseline: 1.1599x; 1.0091x over previous
"""MHA kernel builder for TRN2 (per-core SPMD program) + host prep.

Problem: out = X + MHA(RMSNorm(X)) where Q=K=V=(RMSNorm(X)@Wq.T+b), rope,
causal softmax, Wo projection. Sharding: batch(2) x head-groups(4) over 8
cores; each core computes a partial of out[b] (its 4 heads through Wo);
host sums partials + bias + residual.

v2: fp16 everywhere on the PE path, software-pipelined attention emission
(scores/exp of block i interleaved with dn/AV of block i-1 and Wo of the
previous q-chunk), row reciprocals via the fast custom-DVE op, fp16 output.
"""
import math
import itertools
import numpy as np
from contextlib import ExitStack

import concourse.bass as bass
import concourse.mybir as mybir
import concourse.tile as tile

F32 = mybir.dt.float32
F32R = mybir.dt.float32r
F16 = mybir.dt.float16
BF16 = mybir.dt.bfloat16

EPS = float(np.finfo(np.float32).eps)
ROPE_BASE = 10000.0

_ctr = itertools.count()


def legalize_sync_waits(nc, max_waits=1):
    """This walrus accepts at most one sync-wait per instruction; hoist
    excess waits onto same-engine NOPs inserted just before."""
    n_fixed = 0
    for f in nc.m.functions:
        for bb in f.blocks:
            insts = bb.instructions
            out = []
            dirty = False
            for inst in insts:
                si = getattr(inst, "sync_info", None)
                if si is not None and si.on_wait and len(si.on_wait) > max_waits:
                    waits = list(si.on_wait)
                    for w in waits[:-max_waits]:
                        nop = mybir.InstNoOp(
                            name=f"I-syncfix-{next(_ctr)}", engine=inst.engine
                        )
                        nop.sync_info = mybir.SyncInfo(on_wait=[w], on_update=[])
                        nc.register_instruction(nop, overwrite=True)
                        out.append(nop)
                    inst.sync_info = mybir.SyncInfo(
                        on_wait=waits[-max_waits:], on_update=list(si.on_update or [])
                    )
                    dirty = True
                    n_fixed += 1
                out.append(inst)
            if dirty:
                bb.instructions = out
    return n_fixed


def build_core(S=2048, D=2048, NHL=4, DK=128, SHIFT=10.0, debug=False):
    """Emit the per-core program. Returns nc. All cores run this same NEFF
    with different input data."""
    assert S % 512 == 0 and D % 128 == 0 and DK == 128
    SK = S // 512     # 512-wide seq chunks
    KT = D // 128     # contraction tiles for projections
    ST = S // 128     # 128-wide seq tiles
    ML = NHL * DK     # local model width (q columns this core owns)

    nc = bass.Bass("TRN2", num_devices=8)
    dXT = nc.dram_tensor("XT", [D, S], F16, kind="ExternalInput")
    dWQT = nc.dram_tensor("WQT", [D, ML], F16, kind="ExternalInput")
    dWOT = nc.dram_tensor("WOT", [ML, D], F16, kind="ExternalInput")
    dQB = nc.dram_tensor("QB", [128, NHL], F32, kind="ExternalInput")
    dCOSA = nc.dram_tensor("COSA", [DK, S], F16, kind="ExternalInput")
    dSINA = nc.dram_tensor("SINA", [DK, S], F16, kind="ExternalInput")
    dTRI = nc.dram_tensor("TRI", [128, 128], F16, kind="ExternalInput")
    dONEC = nc.dram_tensor("ONEC", [128, 1], F32, kind="ExternalInput")
    dONER = nc.dram_tensor("ONER", [1, 128], F32, kind="ExternalInput")
    dIDT = nc.dram_tensor("IDT", [128, 128], F16, kind="ExternalInput")
    dOUT = nc.dram_tensor("OUTP", [S, D], F16, kind="ExternalOutput")
    if debug:
        dDBG_Q = nc.dram_tensor("DBG_Q", [DK, S], F32, kind="ExternalOutput")
        dDBG_R = nc.dram_tensor("DBG_R", [1, S], F32, kind="ExternalOutput")
        dDBG_S = nc.dram_tensor("DBG_S", [1, S], F32, kind="ExternalOutput")
        dDBG_M = nc.dram_tensor("DBG_M", [128, S // 128], F32,
                                kind="ExternalOutput")

    with tile.TileContext(nc) as tc, ExitStack() as ctx:
        pp = ctx.enter_context(tc.tile_pool(name="pp", bufs=1))

        # ---- constants (whole-kernel scope) ------------------------------
        tri = pp.tile([128, 128], BF16, name="tri")
        onecf = pp.tile([128, 1], F16, name="onecf")
        onecb = pp.tile([128, 1], BF16, name="onecb")
        oner16 = pp.tile([1, 128], F16, name="oner16")
        onerR = pp.tile([1, 128], F32R, name="onerR")
        idt = pp.tile([128, 128], F16, name="idt")
        qb = pp.tile([128, NHL], F32, name="qb")
        shift_t = pp.tile([128, 1], F32, name="shift_t")
        eps_t = pp.tile([128, 1], F32, name="eps_t")
        nc.gpsimd.dma_start(out=tri, in_=dTRI[:, :])
        nc.gpsimd.dma_start(out=onecf, in_=dONEC[:, :])
        nc.gpsimd.dma_start(out=onecb, in_=dONEC[:, :])
        nc.gpsimd.dma_start(out=oner16, in_=dONER[:, :])
        nc.gpsimd.dma_start(out=onerR, in_=dONER[:, :])
        nc.gpsimd.dma_start(out=idt, in_=dIDT[:, :])
        nc.gpsimd.dma_start(out=qb, in_=dQB[:, :])
        nc.vector.memset(shift_t, -SHIFT)
        nc.vector.memset(eps_t, EPS)

        # persistent across phases: rope'd Q, V tiles, attention out, Wo w.
        # qth/cosa/sina are persistent too so the tail head's rope (DVE)
        # doesn't gate the phase-B pool allocations (SBUF reuse barrier).
        pmid = ctx.enter_context(tc.tile_pool(name="pmid", bufs=1))
        qr = [pmid.tile([DK, S], F16, name=f"qr{h}") for h in range(NHL)]
        vv = [pmid.tile([128, S], BF16, name=f"vv{h}") for h in range(NHL)]
        wo = pmid.tile([128, NHL, D], F16, name="wo")
        qth = [pmid.tile([DK, S], F16, name=f"qth{h}") for h in range(NHL)]
        cosa = pmid.tile([DK, S], F16, name="cosa")
        sina = pmid.tile([DK, S], F16, name="sina")

        # =================== Phase A: load, RMS, Q proj ===================
        with tc.tile_pool(name="pxw", bufs=1) as pxw, \
             tc.tile_pool(name="psQ", bufs=3, space="PSUM") as psQ, \
             tc.tile_pool(name="psRow", bufs=4, space="PSUM") as psRow, \
             tc.tile_pool(name="psAux", bufs=1, space="PSUM") as psAux:

            xt = [pxw.tile([128, S], F16, name=f"xt{k}") for k in range(KT)]
            wq = pxw.tile([128, KT, ML], F16, name="wq")

            for k in range(KT):
                nc.sync.dma_start(out=xt[k], in_=dXT[k * 128:(k + 1) * 128, :])
                nc.sync.dma_start(out=wq[:, k, :],
                                  in_=dWQT[k * 128:(k + 1) * 128, :])

            # Q-proj combos; first wave overlaps the ssq/DMA window
            combos = [(mt, c) for mt in range(NHL) for c in range(SK)]
            wave0, rest = combos[:3], combos[3:]
            q_ps = {}
            for mt, c in wave0:
                q_ps[(mt, c)] = psQ.tile([128, 512], F32,
                                         name=f"qps{mt}_{c}", tag="q")

            # ssq rows: 4 x [1,512] PSUM rows (one bank each)
            ssq_ps = [psRow.tile([1, 512], F32, name=f"ssq{c}", tag="row")
                      for c in range(SK)]
            for k in range(KT):
                sq = pxw.tile([128, S], F16, name=f"sq{k}", tag="sq", bufs=2)
                nc.vector.tensor_mul(sq, xt[k], xt[k])
                for mt, c in wave0:
                    nc.tensor.matmul(
                        q_ps[(mt, c)],
                        wq[:, k, mt * 128:(mt + 1) * 128],
                        xt[k][:, c * 512:(c + 1) * 512],
                        start=(k == 0), stop=(k == KT - 1))
                for c in range(SK):
                    nc.tensor.matmul(ssq_ps[c], onecf,
                                     sq[:, c * 512:(c + 1) * 512],
                                     start=(k == 0), stop=(k == KT - 1))
            # bulk loads needed later: same queue as xt/wq so their
            # transfers don't steal HBM bandwidth from the critical xt path
            nc.sync.dma_start(out=cosa, in_=dCOSA[:, :])
            nc.sync.dma_start(out=sina, in_=dSINA[:, :])
            for h in range(NHL):
                nc.sync.dma_start(out=wo[:, h, :],
                                  in_=dWOT[h * 128:(h + 1) * 128, :])

            # rms chain: rows -> [128,16] -> rsqrt -> back to rows -> Rbc
            sqrow = pxw.tile([1, S], F32, name="sqrow")
            for c in range(SK):
                nc.vector.tensor_copy(sqrow[0:1, c * 512:(c + 1) * 512],
                                      ssq_ps[c])
            onef32 = pxw.tile([128, 1], F32, name="onef32")
            nc.gpsimd.dma_start(out=onef32, in_=dONEC[:, :])
            auxa = psAux.tile([128, 512], F32, name="auxa", tag="aux")
            for c in range(SK):
                for j in range(4):
                    t = c * 4 + j
                    nc.tensor.transpose(
                        auxa[:, t:t + 1],
                        sqrow[0:1, c * 512 + j * 128:c * 512 + (j + 1) * 128],
                        onef32[0:1, 0:1])
            ms128 = pxw.tile([128, ST], F32, name="ms128")
            nc.vector.tensor_copy(ms128, auxa[:, 0:ST])
            # r = 1/sqrt(ms/D + eps) = exp(-0.5 * ln(ms/D + eps))
            lnms = pxw.tile([128, ST], F32, name="lnms")
            nc.scalar.activation(lnms, ms128,
                                 mybir.ActivationFunctionType.Ln,
                                 bias=eps_t, scale=1.0 / D)
            r128 = pxw.tile([128, ST], F16, name="r128")
            nc.scalar.activation(r128, lnms,
                                 mybir.ActivationFunctionType.Exp,
                                 scale=-0.5)
            # transpose back column-by-column into a partition-0 row
            rrow = pxw.tile([1, S], F16, name="rrow")
            auxb = psAux.tile([128, 1024], F16, name="auxb", tag="aux")
            for c in range(SK):
                for j in range(4):
                    t = c * 4 + j
                    nc.tensor.transpose(
                        auxb[0:1, j * 128:(j + 1) * 128],
                        r128[:, t:t + 1], idt)
                nc.vector.tensor_copy(rrow[0:1, c * 512:(c + 1) * 512],
                                      auxb[0:1, 0:512])

            # Rbc: broadcast r across partitions, [128, S] f16 in SBUF
            Rbc = pxw.tile([128, S], F16, name="Rbc")
            for c in range(SK):
                rbc_ps = psRow.tile([128, 512], F32, name=f"rbc{c}", tag="row")
                for j in range(4):
                    sl = slice(c * 512 + j * 128, c * 512 + (j + 1) * 128)
                    nc.tensor.matmul(rbc_ps[:, j * 128:(j + 1) * 128], oner16,
                                     rrow[0:1, sl],
                                     start=True, stop=True)
                nc.vector.tensor_copy(Rbc[:, c * 512:(c + 1) * 512], rbc_ps)

            # Q projection drains + remaining combos; rope + V per done head
            def drain_combo(mt, c):
                sl = slice(c * 512, (c + 1) * 512)
                nc.vector.tensor_mul(qth[mt][:, sl], q_ps[(mt, c)], Rbc[:, sl])
                nc.vector.tensor_scalar_add(qth[mt][:, sl], qth[mt][:, sl],
                                            qb[:, mt:mt + 1])

            def finish_head(mt):
                # V = qth.T per head (PE transpose via identity); first so
                # the psRow banks free before the rope runs on DVE
                for g in range(ST // 4):
                    vaux = psRow.tile([128, 512], F16, name=f"vx{mt}_{g}",
                                      tag="row")
                    for j in range(4):
                        st = g * 4 + j
                        nc.tensor.transpose(
                            vaux[:, j * 128:(j + 1) * 128],
                            qth[mt][:, st * 128:(st + 1) * 128], idt)
                    nc.vector.tensor_copy(vv[mt][:, g * 512:(g + 1) * 512],
                                          vaux)
                # rope for this head (reads persistent tiles only)
                sh = pmid.tile([DK, S], F16, name=f"sh{mt}", tag="ropesh",
                               bufs=2)
                hw = DK // 2
                nc.vector.tensor_copy(sh[0:hw, :], qth[mt][hw:DK, :])
                nc.vector.tensor_copy(sh[hw:DK, :], qth[mt][0:hw, :])
                m1 = pmid.tile([DK, S], F16, name=f"m1_{mt}", tag="ropem1",
                               bufs=2)
                nc.vector.tensor_mul(m1, qth[mt], cosa)
                nc.vector.tensor_mul(sh, sh, sina)
                nc.vector.tensor_add(qr[mt], m1, sh)

            for mt, c in wave0:
                drain_combo(mt, c)
            for mt, c in rest:
                q_ps[(mt, c)] = psQ.tile([128, 512], F32,
                                         name=f"qps{mt}_{c}", tag="q")
                for k in range(KT):
                    nc.tensor.matmul(
                        q_ps[(mt, c)],
                        wq[:, k, mt * 128:(mt + 1) * 128],
                        xt[k][:, c * 512:(c + 1) * 512],
                        start=(k == 0), stop=(k == KT - 1))
                drain_combo(mt, c)
                if c == SK - 1:
                    finish_head(mt)

            if debug:
                nc.gpsimd.dma_start(out=dDBG_Q[:, :], in_=qth[0][:, :])
                nc.gpsimd.dma_start(out=dDBG_R[:, :], in_=rrow[:, :])
                nc.gpsimd.dma_start(out=dDBG_S[:, :], in_=sqrow[:, :])
                nc.gpsimd.dma_start(out=dDBG_M[:, :], in_=ms128[:, :])

        # =================== Phase B: attention + Wo ======================
        with tc.tile_pool(name="patt", bufs=1) as pat, \
             tc.tile_pool(name="psS", bufs=3, space="PSUM") as psS, \
             tc.tile_pool(name="psO", bufs=2, space="PSUM") as psO, \
             tc.tile_pool(name="psDn", bufs=1, space="PSUM") as psDn, \
             tc.tile_pool(name="psW", bufs=2, space="PSUM") as psW:

            ot = [pat.tile([DK, S], F16, name=f"ot{h}") for h in range(NHL)]
            NC = D // 512

            # per-block state carried between pipeline stages
            def new_block(h, qc):
                npt = min(ST, qc * 4 + 4)
                return {
                    "h": h, "qc": qc, "npt": npt,
                    "atq": [pat.tile([128, 512], BF16, name=f"at{h}_{qc}_{p}",
                                     tag=f"atq{p}", bufs=3)
                            for p in range(npt)],
                    "dn_ps": None,
                    "o_ps": None,
                }

            def emit_score(blk, pt):
                h, qc = blk["h"], blk["qc"]
                off = pt * 128 - qc * 512
                lo = max(0, off)
                n = 512 - lo
                s_ps = psS.tile([128, 512], F32, name=f"sps{h}_{qc}_{pt}",
                                tag="s")
                nc.tensor.matmul(
                    s_ps[:, 0:n],
                    qr[h][:, pt * 128:(pt + 1) * 128],
                    qr[h][:, qc * 512 + lo:(qc + 1) * 512],
                    start=True, stop=True)
                nc.scalar.activation(blk["atq"][pt][:, lo:512], s_ps[:, 0:n],
                                     mybir.ActivationFunctionType.Exp,
                                     bias=shift_t)
                if off >= 0:
                    # diagonal block: triangular causal mask (keep p <= q)
                    nc.vector.tensor_mul(blk["atq"][pt][:, lo:lo + 128],
                                         blk["atq"][pt][:, lo:lo + 128], tri)

            def emit_dn(blk, pt):
                off = pt * 128 - blk["qc"] * 512
                lo = max(0, off)
                if blk["dn_ps"] is None:
                    blk["dn_ps"] = psDn.tile([1, 512], F32,
                                             name=f"dn{blk['h']}_{blk['qc']}",
                                             tag="dn")
                nc.tensor.matmul(blk["dn_ps"][0:1, lo:512],
                                 onecb, blk["atq"][pt][:, lo:512],
                                 start=(pt == 0), stop=(pt == blk["npt"] - 1),
                                 skip_group_check=True)

            def emit_av(blk, pt):
                h, qc = blk["h"], blk["qc"]
                off = pt * 128 - qc * 512
                lo = max(0, off)
                if blk["o_ps"] is None:
                    blk["o_ps"] = psO.tile([128, 512], F32, name=f"o{h}_{qc}",
                                           tag="o")
                nc.tensor.matmul(blk["o_ps"][:, lo:512],
                                 vv[h][:, pt * 128:(pt + 1) * 128],
                                 blk["atq"][pt][:, lo:512],
                                 start=(pt == 0), stop=(pt == blk["npt"] - 1),
                                 skip_group_check=True)

            def emit_recip(blk):
                h, qc = blk["h"], blk["qc"]
                # 1/dn = exp(-ln(dn)) on the Act engine (PSUM read, same
                # table set as the attention exp)
                lnrow = pat.tile([1, 512], F32, name=f"lnr{h}_{qc}",
                                 tag="lnrow", bufs=2)
                nc.scalar.activation(lnrow, blk["dn_ps"],
                                     mybir.ActivationFunctionType.Ln)
                rdrow_r = pat.tile([1, 512], F32R, name=f"rdh{h}_{qc}",
                                   tag="rdrow16", bufs=2)
                nc.scalar.activation(rdrow_r, lnrow,
                                     mybir.ActivationFunctionType.Exp,
                                     scale=-1.0)
                bc2 = psW.tile([128, 512], F32, name=f"bc2{h}_{qc}", tag="w")
                nc.tensor.matmul(bc2, onerR, rdrow_r,
                                 start=True, stop=True)
                blk["bc2"] = bc2

            def emit_norm2(blk):
                h, qc = blk["h"], blk["qc"]
                rdb = pat.tile([128, 512], F32, name=f"rdb{h}_{qc}",
                               tag="rdb", bufs=2)
                nc.vector.tensor_copy(rdb, blk["bc2"])
                sl = slice(qc * 512, (qc + 1) * 512)
                nc.vector.tensor_mul(ot[h][:, sl], blk["o_ps"], rdb)

            # Wo work queue: one st-group = 4 ncc PSUM groups + copies + DMA.
            # Copies alternate DVE/Act and the output DMA goes in halves so
            # the drain tail stays short.
            def emit_wo_group(st, engine_mix=False):
                out_sb = pat.tile([128, D], F16, name=f"osb{st}", tag="osb",
                                  bufs=2)
                for ncc in range(NC):
                    wo_ps = psW.tile([128, 512], F32, name=f"wops{st}_{ncc}",
                                     tag="w")
                    for h in range(NHL):
                        nc.tensor.matmul(
                            wo_ps,
                            ot[h][:, st * 128:(st + 1) * 128],
                            wo[:, h, ncc * 512:(ncc + 1) * 512],
                            start=(h == 0), stop=(h == NHL - 1))
                    csl = slice(ncc * 512, (ncc + 1) * 512)
                    if engine_mix and ncc % 2 == 1:
                        nc.scalar.copy(out_sb[:, csl], wo_ps)
                    else:
                        nc.vector.tensor_copy(out_sb[:, csl], wo_ps)
                    if ncc == 1:
                        nc.sync.dma_start(
                            out=dOUT[st * 128:(st + 1) * 128, 0:1024],
                            in_=out_sb[:, 0:1024])
                nc.sync.dma_start(out=dOUT[st * 128:(st + 1) * 128, 1024:D],
                                  in_=out_sb[:, 1024:D])

            # depth-2 software pipeline: scores/exp of block i interleaved
            # with dn/AV of block i-2, so PE never waits on the Act engine
            prev = prev2 = None
            wo_pending = []   # st indices ready to emit
            for qc in range(SK):
                for h in range(NHL):
                    cur = new_block(h, qc)
                    np_c = cur["npt"]
                    np_p = prev2["npt"] if prev2 else 0
                    n = max(np_c, np_p)
                    for i in range(n):
                        if i < np_c:
                            emit_score(cur, i)
                        if prev2 and i < np_p:
                            emit_dn(prev2, i)
                            emit_av(prev2, i)
                            if i == np_p - 1:
                                emit_recip(prev2)
                        # sprinkle one Wo group every ~5 iterations
                        if wo_pending and i % 5 == 4:
                            emit_wo_group(wo_pending.pop(0))
                    if prev2:
                        emit_norm2(prev2)
                        if prev2["h"] == NHL - 1:
                            # all heads of prev2's qc normalized -> Wo ready
                            wo_pending.extend(range(prev2["qc"] * 4,
                                                    prev2["qc"] * 4 + 4))
                    if wo_pending:
                        emit_wo_group(wo_pending.pop(0))
                    prev2, prev = prev, cur

            # drain the two in-flight blocks + remaining Wo groups
            for blk in (prev2, prev):
                for i in range(blk["npt"]):
                    emit_dn(blk, i)
                    emit_av(blk, i)
                    if wo_pending and i % 5 == 4:
                        emit_wo_group(wo_pending.pop(0))
                emit_recip(blk)
                emit_norm2(blk)
                if blk["h"] == NHL - 1:
                    wo_pending.extend(range(blk["qc"] * 4, blk["qc"] * 4 + 4))
            while wo_pending:
                emit_wo_group(wo_pending.pop(0), engine_mix=True)

    return nc


# ======================= host-side preparation ===========================

def host_prep(X, Wq_w, Wq_b, Wo_w, Wo_b, rms_w, n_cores=8, NHL=4):
    """Build per-core input maps. X: (B,S,D) fp32."""
    B, S, D = X.shape
    DK = 128
    c = DK ** -0.25
    inv = 1.0 / (ROPE_BASE ** (np.arange(0, DK, 2, dtype=np.float64) / DK))
    ang = np.arange(S, dtype=np.float64)[:, None] * inv[None, :]
    cos = np.concatenate([np.cos(ang), np.cos(ang)], -1)     # (S, DK)
    sin = np.concatenate([np.sin(ang), np.sin(ang)], -1)
    COSA = (cos.T * c).astype(np.float16)                    # (DK, S)
    SINT = (sin.T * c).astype(np.float32)
    SINA = np.concatenate([-SINT[:DK // 2], SINT[DK // 2:]], 0).astype(np.float16)
    TRI = np.triu(np.ones((128, 128), np.float32)).astype(np.float16)  # p <= q
    ONEC = np.ones((128, 1), np.float32)
    ONER = np.ones((1, 128), np.float32)
    IDT = np.eye(128, dtype=np.float16)

    Wq_eff = (Wq_w * rms_w[None, :]).astype(np.float32)       # (D, D) fold rms
    in_maps = []
    groups = n_cores // B                                     # head-groups per batch
    ML = NHL * DK
    for core in range(n_cores):
        b = core // groups
        hg = core % groups
        msl = slice(hg * ML, (hg + 1) * ML)
        XT = np.ascontiguousarray(X[b].T).astype(np.float16)          # (D, S)
        WQT = np.ascontiguousarray(Wq_eff[msl, :].T).astype(np.float16)   # (D, ML)
        WOT = np.ascontiguousarray(Wo_w[:, msl].T).astype(np.float16)     # (ML, D)
        QB = np.ascontiguousarray(
            Wq_b[msl].reshape(NHL, 128).T).astype(np.float32)             # (128, NHL)
        in_maps.append({
            "XT": XT, "WQT": WQT, "WOT": WOT, "QB": QB,
            "COSA": COSA, "SINA": SINA, "TRI": TRI,
            "ONEC": ONEC, "ONER": ONER, "IDT": IDT,
        })
    return in_maps


def host_reduce(X, Wo_b, results, n_cores=8):
    B, S, D = X.shape
    groups = n_cores // B
    out = np.empty((B, S, D), np.float32)
    for b in range(B):
        acc = X[b].astype(np.float32).copy()
        for hg in range(groups):
            acc += results[b * groups + hg]["OUTP"].astype(np.float32)
        acc += Wo_b[None, :]
        out[b] = acc
    return out


# ======================= public entry point ==============================

_CACHE = {}


def _get_nc():
    if "nc" not in _CACHE:
        nc = build_core(S=2048, D=2048, NHL=4, DK=128, SHIFT=10.0)
        legalize_sync_waits(nc, max_waits=1)
        _CACHE["nc"] = nc
    return _CACHE["nc"]


def kernel(X, Wq_w, Wq_b, Wo_w, Wo_b, rms_w):
    """Full-input MHA block: returns X + MHA(RMSNorm(X)) as np.float32.

    Shards batch(2) x head-groups(4) across 8 NeuronCores; each core
    produces a partial output (its 4 heads through Wo); the host sums the
    four partials per batch and adds bias + residual.
    """
    from concourse.bass_utils import run_bass_kernel_spmd

    X = np.asarray(X, np.float32)
    Wq_w = np.asarray(Wq_w, np.float32)
    Wq_b = np.asarray(Wq_b, np.float32)
    Wo_w = np.asarray(Wo_w, np.float32)
    Wo_b = np.asarray(Wo_b, np.float32)
    rms_w = np.asarray(rms_w, np.float32)

    nc = _get_nc()
    in_maps = host_prep(X, Wq_w, Wq_b, Wo_w, Wo_b, rms_w)
    res = run_bass_kernel_spmd(nc, in_maps, core_ids=list(range(8)))
    return host_reduce(X, Wo_b, res.results)


# revision 58
# speedup vs baseline: 1.1694x; 1.0081x over previous
"""MHA kernel builder for TRN2 (per-core SPMD program) + host prep.

Problem: out = X + MHA(RMSNorm(X)) where Q=K=V=(RMSNorm(X)@Wq.T+b), rope,
causal softmax, Wo projection. Sharding: batch(2) x head-groups(4) over 8
cores; each core computes a partial of out[b] (its 4 heads through Wo);
host sums partials + bias + residual.

v2: fp16 everywhere on the PE path, software-pipelined attention emission
(scores/exp of block i interleaved with dn/AV of block i-1 and Wo of the
previous q-chunk), row reciprocals via the fast custom-DVE op, fp16 output.
"""
import math
import itertools
import numpy as np
from contextlib import ExitStack

import concourse.bass as bass
import concourse.mybir as mybir
import concourse.tile as tile

F32 = mybir.dt.float32
F32R = mybir.dt.float32r
F16 = mybir.dt.float16
BF16 = mybir.dt.bfloat16

EPS = float(np.finfo(np.float32).eps)
ROPE_BASE = 10000.0

_ctr = itertools.count()


def legalize_sync_waits(nc, max_waits=1):
    """This walrus accepts at most one sync-wait per instruction; hoist
    excess waits onto same-engine NOPs inserted just before."""
    n_fixed = 0
    for f in nc.m.functions:
        for bb in f.blocks:
            insts = bb.instructions
            out = []
            dirty = False
            for inst in insts:
                si = getattr(inst, "sync_info", None)
                if si is not None and si.on_wait and len(si.on_wait) > max_waits:
                    waits = list(si.on_wait)
                    for w in waits[:-max_waits]:
                        nop = mybir.InstNoOp(
                            name=f"I-syncfix-{next(_ctr)}", engine=inst.engine
                        )
                        nop.sync_info = mybir.SyncInfo(on_wait=[w], on_update=[])
                        nc.register_instruction(nop, overwrite=True)
                        out.append(nop)
                    inst.sync_info = mybir.SyncInfo(
                        on_wait=waits[-max_waits:], on_update=list(si.on_update or [])
                    )
                    dirty = True
                    n_fixed += 1
                out.append(inst)
            if dirty:
                bb.instructions = out
    return n_fixed


def build_core(S=2048, D=2048, NHL=4, DK=128, SHIFT=10.0, debug=False):
    """Emit the per-core program. Returns nc. All cores run this same NEFF
    with different input data."""
    assert S % 512 == 0 and D % 128 == 0 and DK == 128
    SK = S // 512     # 512-wide seq chunks
    KT = D // 128     # contraction tiles for projections
    ST = S // 128     # 128-wide seq tiles
    ML = NHL * DK     # local model width (q columns this core owns)

    nc = bass.Bass("TRN2", num_devices=8)
    dXT = nc.dram_tensor("XT", [D, S], F16, kind="ExternalInput")
    dWQT = nc.dram_tensor("WQT", [D, ML], F16, kind="ExternalInput")
    dWOT = nc.dram_tensor("WOT", [ML, D], F16, kind="ExternalInput")
    dQB = nc.dram_tensor("QB", [128, NHL], F32, kind="ExternalInput")
    dCOSA = nc.dram_tensor("COSA", [DK, S], F16, kind="ExternalInput")
    dSINA = nc.dram_tensor("SINA", [DK, S], F16, kind="ExternalInput")
    dTRI = nc.dram_tensor("TRI", [128, 128], F16, kind="ExternalInput")
    dONEC = nc.dram_tensor("ONEC", [128, 1], F32, kind="ExternalInput")
    dONER = nc.dram_tensor("ONER", [1, 128], F32, kind="ExternalInput")
    dIDT = nc.dram_tensor("IDT", [128, 128], F16, kind="ExternalInput")
    dOUT = nc.dram_tensor("OUTP", [S, D], F16, kind="ExternalOutput")
    if debug:
        dDBG_Q = nc.dram_tensor("DBG_Q", [DK, S], F32, kind="ExternalOutput")
        dDBG_R = nc.dram_tensor("DBG_R", [1, S], F32, kind="ExternalOutput")
        dDBG_S = nc.dram_tensor("DBG_S", [1, S], F32, kind="ExternalOutput")
        dDBG_M = nc.dram_tensor("DBG_M", [128, S // 128], F32,
                                kind="ExternalOutput")

    with tile.TileContext(nc) as tc, ExitStack() as ctx:
        pp = ctx.enter_context(tc.tile_pool(name="pp", bufs=1))

        # ---- constants (whole-kernel scope) ------------------------------
        tri = pp.tile([128, 128], BF16, name="tri")
        onecf = pp.tile([128, 1], F16, name="onecf")
        onecb = pp.tile([128, 1], BF16, name="onecb")
        oner16 = pp.tile([1, 128], F16, name="oner16")
        onerB = pp.tile([1, 128], BF16, name="onerB")
        idt = pp.tile([128, 128], F16, name="idt")
        qb = pp.tile([128, NHL], F32, name="qb")
        shift_t = pp.tile([128, 1], F32, name="shift_t")
        eps_t = pp.tile([128, 1], F32, name="eps_t")
        nc.gpsimd.dma_start(out=tri, in_=dTRI[:, :])
        nc.gpsimd.dma_start(out=onecf, in_=dONEC[:, :])
        nc.gpsimd.dma_start(out=onecb, in_=dONEC[:, :])
        nc.gpsimd.dma_start(out=oner16, in_=dONER[:, :])
        nc.gpsimd.dma_start(out=onerB, in_=dONER[:, :])
        nc.gpsimd.dma_start(out=idt, in_=dIDT[:, :])
        nc.gpsimd.dma_start(out=qb, in_=dQB[:, :])
        nc.vector.memset(shift_t, -SHIFT)
        nc.vector.memset(eps_t, EPS)

        # persistent across phases: rope'd Q, V tiles, attention out, Wo w.
        # qth/cosa/sina are persistent too so the tail head's rope (DVE)
        # doesn't gate the phase-B pool allocations (SBUF reuse barrier).
        pmid = ctx.enter_context(tc.tile_pool(name="pmid", bufs=1))
        qr = [pmid.tile([DK, S], F16, name=f"qr{h}") for h in range(NHL)]
        vv = [pmid.tile([128, S], BF16, name=f"vv{h}") for h in range(NHL)]
        wo = pmid.tile([128, NHL, D], F16, name="wo")
        qth = [pmid.tile([DK, S], F16, name=f"qth{h}") for h in range(NHL)]
        cosa = pmid.tile([DK, S], F16, name="cosa")
        sina = pmid.tile([DK, S], F16, name="sina")

        # =================== Phase A: load, RMS, Q proj ===================
        with tc.tile_pool(name="pxw", bufs=1) as pxw, \
             tc.tile_pool(name="psQ", bufs=3, space="PSUM") as psQ, \
             tc.tile_pool(name="psRow", bufs=4, space="PSUM") as psRow, \
             tc.tile_pool(name="psAux", bufs=1, space="PSUM") as psAux:

            xt = [pxw.tile([128, S], F16, name=f"xt{k}") for k in range(KT)]
            wq = pxw.tile([128, KT, ML], F16, name="wq")

            for k in range(KT):
                nc.sync.dma_start(out=xt[k], in_=dXT[k * 128:(k + 1) * 128, :])
                nc.sync.dma_start(out=wq[:, k, :],
                                  in_=dWQT[k * 128:(k + 1) * 128, :])

            # Q-proj combos; first wave overlaps the ssq/DMA window
            combos = [(mt, c) for mt in range(NHL) for c in range(SK)]
            wave0, rest = combos[:3], combos[3:]
            q_ps = {}
            for mt, c in wave0:
                q_ps[(mt, c)] = psQ.tile([128, 512], F32,
                                         name=f"qps{mt}_{c}", tag="q")

            # ssq rows: 4 x [1,512] PSUM rows (one bank each)
            ssq_ps = [psRow.tile([1, 512], F32, name=f"ssq{c}", tag="row")
                      for c in range(SK)]
            for k in range(KT):
                sq = pxw.tile([128, S], F16, name=f"sq{k}", tag="sq", bufs=2)
                nc.vector.tensor_mul(sq, xt[k], xt[k])
                for mt, c in wave0:
                    nc.tensor.matmul(
                        q_ps[(mt, c)],
                        wq[:, k, mt * 128:(mt + 1) * 128],
                        xt[k][:, c * 512:(c + 1) * 512],
                        start=(k == 0), stop=(k == KT - 1))
                for c in range(SK):
                    nc.tensor.matmul(ssq_ps[c], onecf,
                                     sq[:, c * 512:(c + 1) * 512],
                                     start=(k == 0), stop=(k == KT - 1))
            # bulk loads needed later: same queue as xt/wq so their
            # transfers don't steal HBM bandwidth from the critical xt path
            nc.sync.dma_start(out=cosa, in_=dCOSA[:, :])
            nc.sync.dma_start(out=sina, in_=dSINA[:, :])
            for h in range(NHL):
                nc.sync.dma_start(out=wo[:, h, :],
                                  in_=dWOT[h * 128:(h + 1) * 128, :])

            # rms chain: rows -> [128,16] -> rsqrt -> back to rows -> Rbc
            sqrow = pxw.tile([1, S], F32, name="sqrow")
            for c in range(SK):
                nc.vector.tensor_copy(sqrow[0:1, c * 512:(c + 1) * 512],
                                      ssq_ps[c])
            onef32 = pxw.tile([128, 1], F32, name="onef32")
            nc.gpsimd.dma_start(out=onef32, in_=dONEC[:, :])
            auxa = psAux.tile([128, 512], F32, name="auxa", tag="aux")
            for c in range(SK):
                for j in range(4):
                    t = c * 4 + j
                    nc.tensor.transpose(
                        auxa[:, t:t + 1],
                        sqrow[0:1, c * 512 + j * 128:c * 512 + (j + 1) * 128],
                        onef32[0:1, 0:1])
            ms128 = pxw.tile([128, ST], F32, name="ms128")
            nc.vector.tensor_copy(ms128, auxa[:, 0:ST])
            # r = 1/sqrt(ms/D + eps) = exp(-0.5 * ln(ms/D + eps))
            lnms = pxw.tile([128, ST], F32, name="lnms")
            nc.scalar.activation(lnms, ms128,
                                 mybir.ActivationFunctionType.Ln,
                                 bias=eps_t, scale=1.0 / D)
            r128 = pxw.tile([128, ST], F16, name="r128")
            nc.scalar.activation(r128, lnms,
                                 mybir.ActivationFunctionType.Exp,
                                 scale=-0.5)
            # transpose back column-by-column into a partition-0 row
            rrow = pxw.tile([1, S], F16, name="rrow")
            auxb = psAux.tile([128, 1024], F16, name="auxb", tag="aux")
            for c in range(SK):
                for j in range(4):
                    t = c * 4 + j
                    nc.tensor.transpose(
                        auxb[0:1, j * 128:(j + 1) * 128],
                        r128[:, t:t + 1], idt)
                nc.vector.tensor_copy(rrow[0:1, c * 512:(c + 1) * 512],
                                      auxb[0:1, 0:512])

            # Rbc: broadcast r across partitions, [128, S] f16 in SBUF
            Rbc = pxw.tile([128, S], F16, name="Rbc")
            for c in range(SK):
                rbc_ps = psRow.tile([128, 512], F32, name=f"rbc{c}", tag="row")
                for j in range(4):
                    sl = slice(c * 512 + j * 128, c * 512 + (j + 1) * 128)
                    nc.tensor.matmul(rbc_ps[:, j * 128:(j + 1) * 128], oner16,
                                     rrow[0:1, sl],
                                     start=True, stop=True)
                nc.vector.tensor_copy(Rbc[:, c * 512:(c + 1) * 512], rbc_ps)

            # Q projection drains + remaining combos; rope + V per done head
            def drain_combo(mt, c):
                sl = slice(c * 512, (c + 1) * 512)
                nc.vector.tensor_mul(qth[mt][:, sl], q_ps[(mt, c)], Rbc[:, sl])
                nc.vector.tensor_scalar_add(qth[mt][:, sl], qth[mt][:, sl],
                                            qb[:, mt:mt + 1])

            def finish_head(mt):
                # V = qth.T per head (PE transpose via identity); first so
                # the psRow banks free before the rope runs on DVE
                for g in range(ST // 4):
                    vaux = psRow.tile([128, 512], F16, name=f"vx{mt}_{g}",
                                      tag="row")
                    for j in range(4):
                        st = g * 4 + j
                        nc.tensor.transpose(
                            vaux[:, j * 128:(j + 1) * 128],
                            qth[mt][:, st * 128:(st + 1) * 128], idt)
                    nc.vector.tensor_copy(vv[mt][:, g * 512:(g + 1) * 512],
                                          vaux)
                # rope for this head (reads persistent tiles only)
                sh = pmid.tile([DK, S], F16, name=f"sh{mt}", tag="ropesh",
                               bufs=2)
                hw = DK // 2
                nc.vector.tensor_copy(sh[0:hw, :], qth[mt][hw:DK, :])
                nc.vector.tensor_copy(sh[hw:DK, :], qth[mt][0:hw, :])
                m1 = pmid.tile([DK, S], F16, name=f"m1_{mt}", tag="ropem1",
                               bufs=2)
                nc.vector.tensor_mul(m1, qth[mt], cosa)
                nc.vector.tensor_mul(sh, sh, sina)
                nc.vector.tensor_add(qr[mt], m1, sh)

            for mt, c in wave0:
                drain_combo(mt, c)
            for mt, c in rest:
                q_ps[(mt, c)] = psQ.tile([128, 512], F32,
                                         name=f"qps{mt}_{c}", tag="q")
                for k in range(KT):
                    nc.tensor.matmul(
                        q_ps[(mt, c)],
                        wq[:, k, mt * 128:(mt + 1) * 128],
                        xt[k][:, c * 512:(c + 1) * 512],
                        start=(k == 0), stop=(k == KT - 1))
                drain_combo(mt, c)
                if c == SK - 1:
                    finish_head(mt)

            if debug:
                nc.gpsimd.dma_start(out=dDBG_Q[:, :], in_=qth[0][:, :])
                nc.gpsimd.dma_start(out=dDBG_R[:, :], in_=rrow[:, :])
                nc.gpsimd.dma_start(out=dDBG_S[:, :], in_=sqrow[:, :])
                nc.gpsimd.dma_start(out=dDBG_M[:, :], in_=ms128[:, :])

        # =================== Phase B: attention + Wo ======================
        with tc.tile_pool(name="patt", bufs=1) as pat, \
             tc.tile_pool(name="psS", bufs=3, space="PSUM") as psS, \
             tc.tile_pool(name="psO", bufs=2, space="PSUM") as psO, \
             tc.tile_pool(name="psDn", bufs=1, space="PSUM") as psDn, \
             tc.tile_pool(name="psW", bufs=2, space="PSUM") as psW:

            ot = [pat.tile([DK, S], F16, name=f"ot{h}") for h in range(NHL)]
            NC = D // 512

            # per-block state carried between pipeline stages
            def new_block(h, qc):
                npt = min(ST, qc * 4 + 4)
                return {
                    "h": h, "qc": qc, "npt": npt,
                    "atq": [pat.tile([128, 512], BF16, name=f"at{h}_{qc}_{p}",
                                     tag=f"atq{p}", bufs=3)
                            for p in range(npt)],
                    "dn_ps": None,
                    "o_ps": None,
                }

            def emit_score(blk, pt):
                h, qc = blk["h"], blk["qc"]
                off = pt * 128 - qc * 512
                lo = max(0, off)
                n = 512 - lo
                s_ps = psS.tile([128, 512], F32, name=f"sps{h}_{qc}_{pt}",
                                tag="s")
                nc.tensor.matmul(
                    s_ps[:, 0:n],
                    qr[h][:, pt * 128:(pt + 1) * 128],
                    qr[h][:, qc * 512 + lo:(qc + 1) * 512],
                    start=True, stop=True)
                nc.scalar.activation(blk["atq"][pt][:, lo:512], s_ps[:, 0:n],
                                     mybir.ActivationFunctionType.Exp,
                                     bias=shift_t)
                if off >= 0:
                    # diagonal block: triangular causal mask (keep p <= q)
                    nc.vector.tensor_mul(blk["atq"][pt][:, lo:lo + 128],
                                         blk["atq"][pt][:, lo:lo + 128], tri)

            def emit_dn(blk, pt):
                off = pt * 128 - blk["qc"] * 512
                lo = max(0, off)
                if blk["dn_ps"] is None:
                    blk["dn_ps"] = psDn.tile([1, 512], F32,
                                             name=f"dn{blk['h']}_{blk['qc']}",
                                             tag="dn")
                nc.tensor.matmul(blk["dn_ps"][0:1, lo:512],
                                 onecb, blk["atq"][pt][:, lo:512],
                                 start=(pt == 0), stop=(pt == blk["npt"] - 1),
                                 skip_group_check=True)

            def emit_av(blk, pt):
                h, qc = blk["h"], blk["qc"]
                off = pt * 128 - qc * 512
                lo = max(0, off)
                if blk["o_ps"] is None:
                    blk["o_ps"] = psO.tile([128, 512], F32, name=f"o{h}_{qc}",
                                           tag="o")
                nc.tensor.matmul(blk["o_ps"][:, lo:512],
                                 vv[h][:, pt * 128:(pt + 1) * 128],
                                 blk["atq"][pt][:, lo:512],
                                 start=(pt == 0), stop=(pt == blk["npt"] - 1),
                                 skip_group_check=True)

            def emit_recip(blk):
                h, qc = blk["h"], blk["qc"]
                # 1/dn = exp(-ln(dn)) on the Act engine (PSUM read, same
                # table set as the attention exp)
                lnrow = pat.tile([1, 512], F32, name=f"lnr{h}_{qc}",
                                 tag="lnrow", bufs=2)
                nc.scalar.activation(lnrow, blk["dn_ps"],
                                     mybir.ActivationFunctionType.Ln)
                rdrow_b = pat.tile([1, 512], BF16, name=f"rdh{h}_{qc}",
                                   tag="rdrow16", bufs=2)
                nc.scalar.activation(rdrow_b, lnrow,
                                     mybir.ActivationFunctionType.Exp,
                                     scale=-1.0)
                bc2 = psW.tile([128, 512], F32, name=f"bc2{h}_{qc}", tag="w")
                nc.tensor.matmul(bc2, onerB, rdrow_b,
                                 start=True, stop=True)
                blk["bc2"] = bc2

            def emit_norm2(blk):
                h, qc = blk["h"], blk["qc"]
                rdb = pat.tile([128, 512], F32, name=f"rdb{h}_{qc}",
                               tag="rdb", bufs=2)
                nc.vector.tensor_copy(rdb, blk["bc2"])
                sl = slice(qc * 512, (qc + 1) * 512)
                nc.vector.tensor_mul(ot[h][:, sl], blk["o_ps"], rdb)

            # Wo work queue: one st-group = 4 ncc PSUM groups + copies + DMA.
            # Copies alternate DVE/Act and the output DMA goes in halves so
            # the drain tail stays short.
            def emit_wo_group(st, engine_mix=False):
                out_sb = pat.tile([128, D], F16, name=f"osb{st}", tag="osb",
                                  bufs=2)
                for ncc in range(NC):
                    wo_ps = psW.tile([128, 512], F32, name=f"wops{st}_{ncc}",
                                     tag="w")
                    for h in range(NHL):
                        nc.tensor.matmul(
                            wo_ps,
                            ot[h][:, st * 128:(st + 1) * 128],
                            wo[:, h, ncc * 512:(ncc + 1) * 512],
                            start=(h == 0), stop=(h == NHL - 1))
                    csl = slice(ncc * 512, (ncc + 1) * 512)
                    if engine_mix and ncc % 2 == 1:
                        nc.scalar.copy(out_sb[:, csl], wo_ps)
                    else:
                        nc.vector.tensor_copy(out_sb[:, csl], wo_ps)
                    if ncc == 1:
                        nc.sync.dma_start(
                            out=dOUT[st * 128:(st + 1) * 128, 0:1024],
                            in_=out_sb[:, 0:1024])
                nc.sync.dma_start(out=dOUT[st * 128:(st + 1) * 128, 1024:D],
                                  in_=out_sb[:, 1024:D])

            # depth-2 software pipeline: scores/exp of block i interleaved
            # with dn/AV of block i-2, so PE never waits on the Act engine
            prev = prev2 = None
            wo_pending = []   # st indices ready to emit
            for qc in range(SK):
                for h in range(NHL):
                    cur = new_block(h, qc)
                    np_c = cur["npt"]
                    np_p = prev2["npt"] if prev2 else 0
                    n = max(np_c, np_p)
                    for i in range(n):
                        if i < np_c:
                            emit_score(cur, i)
                        if prev2 and i < np_p:
                            emit_dn(prev2, i)
                            emit_av(prev2, i)
                            if i == np_p - 1:
                                emit_recip(prev2)
                        # sprinkle one Wo group every ~5 iterations
                        if wo_pending and i % 5 == 4:
                            emit_wo_group(wo_pending.pop(0))
                    if prev2:
                        emit_norm2(prev2)
                        if prev2["h"] == NHL - 1:
                            # all heads of prev2's qc normalized -> Wo ready
                            wo_pending.extend(range(prev2["qc"] * 4,
                                                    prev2["qc"] * 4 + 4))
                    if wo_pending:
                        emit_wo_group(wo_pending.pop(0))
                    prev2, prev = prev, cur

            # drain the two in-flight blocks + remaining Wo groups
            for blk in (prev2, prev):
                for i in range(blk["npt"]):
                    emit_dn(blk, i)
                    emit_av(blk, i)
                    if wo_pending and i % 5 == 4:
                        emit_wo_group(wo_pending.pop(0))
                emit_recip(blk)
                emit_norm2(blk)
                if blk["h"] == NHL - 1:
                    wo_pending.extend(range(blk["qc"] * 4, blk["qc"] * 4 + 4))
            while wo_pending:
                emit_wo_group(wo_pending.pop(0), engine_mix=True)

    return nc


# ======================= host-side preparation ===========================

def host_prep(X, Wq_w, Wq_b, Wo_w, Wo_b, rms_w, n_cores=8, NHL=4):
    """Build per-core input maps. X: (B,S,D) fp32."""
    B, S, D = X.shape
    DK = 128
    c = DK ** -0.25
    inv = 1.0 / (ROPE_BASE ** (np.arange(0, DK, 2, dtype=np.float64) / DK))
    ang = np.arange(S, dtype=np.float64)[:, None] * inv[None, :]
    cos = np.concatenate([np.cos(ang), np.cos(ang)], -1)     # (S, DK)
    sin = np.concatenate([np.sin(ang), np.sin(ang)], -1)
    COSA = (cos.T * c).astype(np.float16)                    # (DK, S)
    SINT = (sin.T * c).astype(np.float32)
    SINA = np.concatenate([-SINT[:DK // 2], SINT[DK // 2:]], 0).astype(np.float16)
    TRI = np.triu(np.ones((128, 128), np.float32)).astype(np.float16)  # p <= q
    ONEC = np.ones((128, 1), np.float32)
    ONER = np.ones((1, 128), np.float32)
    IDT = np.eye(128, dtype=np.float16)

    Wq_eff = (Wq_w * rms_w[None, :]).astype(np.float32)       # (D, D) fold rms
    in_maps = []
    groups = n_cores // B                                     # head-groups per batch
    ML = NHL * DK
    for core in range(n_cores):
        b = core // groups
        hg = core % groups
        msl = slice(hg * ML, (hg + 1) * ML)
        XT = np.ascontiguousarray(X[b].T).astype(np.float16)          # (D, S)
        WQT = np.ascontiguousarray(Wq_eff[msl, :].T).astype(np.float16)   # (D, ML)
        WOT = np.ascontiguousarray(Wo_w[:, msl].T).astype(np.float16)     # (ML, D)
        QB = np.ascontiguousarray(
            Wq_b[msl].reshape(NHL, 128).T).astype(np.float32)             # (128, NHL)
        in_maps.append({
            "XT": XT, "WQT": WQT, "WOT": WOT, "QB": QB,
            "COSA": COSA, "SINA": SINA, "TRI": TRI,
            "ONEC": ONEC, "ONER": ONER, "IDT": IDT,
        })
    return in_maps


def host_reduce(X, Wo_b, results, n_cores=8):
    B, S, D = X.shape
    groups = n_cores // B
    out = np.empty((B, S, D), np.float32)
    for b in range(B):
        acc = X[b].astype(np.float32).copy()
        for hg in range(groups):
            acc += results[b * groups + hg]["OUTP"].astype(np.float32)
        acc += Wo_b[None, :]
        out[b] = acc
    return out


# ======================= public entry point ==============================

_CACHE = {}


def _get_nc():
    if "nc" not in _CACHE:
        nc = build_core(S=2048, D=2048, NHL=4, DK=128, SHIFT=10.0)
        legalize_sync_waits(nc, max_waits=1)
        _CACHE["nc"] = nc
    return _CACHE["nc"]


def kernel(X, Wq_w, Wq_b, Wo_w, Wo_b, rms_w):
    """Full-input MHA block: returns X + MHA(RMSNorm(X)) as np.float32.

    Shards batch(2) x head-groups(4) across 8 NeuronCores; each core
    produces a partial output (its 4 heads through Wo); the host sums the
    four partials per batch and adds bias + residual.
    """
    from concourse.bass_utils import run_bass_kernel_spmd

    X = np.asarray(X, np.float32)
    Wq_w = np.asarray(Wq_w, np.float32)
    Wq_b = np.asarray(Wq_b, np.float32)
    Wo_w = np.asarray(Wo_w, np.float32)
    Wo_b = np.asarray(Wo_b, np.float32)
    rms_w = np.asarray(rms_w, np.float32)

    nc = _get_nc()
    in_maps = host_prep(X, Wq_w, Wq_b, Wo_w, Wo_b, rms_w)
    res = run_bass_kernel_spmd(nc, in_maps, core_ids=list(range(8)))
    return host_reduce(X, Wo_b, res.results)


# revision 59
# speedup vs baseline: 1.1859x; 1.0141x over previous
"""MHA kernel builder for TRN2 (per-core SPMD program) + host prep.

Problem: out = X + MHA(RMSNorm(X)) where Q=K=V=(RMSNorm(X)@Wq.T+b), rope,
causal softmax, Wo projection. Sharding: batch(2) x head-groups(4) over 8
cores; each core computes a partial of out[b] (its 4 heads through Wo);
host sums partials + bias + residual.

v2: fp16 everywhere on the PE path, software-pipelined attention emission
(scores/exp of block i interleaved with dn/AV of block i-1 and Wo of the
previous q-chunk), row reciprocals via the fast custom-DVE op, fp16 output.
"""
import math
import itertools
import numpy as np
from contextlib import ExitStack

import concourse.bass as bass
import concourse.mybir as mybir
import concourse.tile as tile

F32 = mybir.dt.float32
F32R = mybir.dt.float32r
F16 = mybir.dt.float16
BF16 = mybir.dt.bfloat16

EPS = float(np.finfo(np.float32).eps)
ROPE_BASE = 10000.0

_ctr = itertools.count()


def legalize_sync_waits(nc, max_waits=1):
    """This walrus accepts at most one sync-wait per instruction; hoist
    excess waits onto same-engine NOPs inserted just before."""
    n_fixed = 0
    for f in nc.m.functions:
        for bb in f.blocks:
            insts = bb.instructions
            out = []
            dirty = False
            for inst in insts:
                si = getattr(inst, "sync_info", None)
                if si is not None and si.on_wait and len(si.on_wait) > max_waits:
                    waits = list(si.on_wait)
                    for w in waits[:-max_waits]:
                        nop = mybir.InstNoOp(
                            name=f"I-syncfix-{next(_ctr)}", engine=inst.engine
                        )
                        nop.sync_info = mybir.SyncInfo(on_wait=[w], on_update=[])
                        nc.register_instruction(nop, overwrite=True)
                        out.append(nop)
                    inst.sync_info = mybir.SyncInfo(
                        on_wait=waits[-max_waits:], on_update=list(si.on_update or [])
                    )
                    dirty = True
                    n_fixed += 1
                out.append(inst)
            if dirty:
                bb.instructions = out
    return n_fixed


def build_core(S=2048, D=2048, NHL=4, DK=128, SHIFT=10.0, debug=False):
    """Emit the per-core program. Returns nc. All cores run this same NEFF
    with different input data."""
    assert S % 512 == 0 and D % 128 == 0 and DK == 128
    SK = S // 512     # 512-wide seq chunks
    KT = D // 128     # contraction tiles for projections
    ST = S // 128     # 128-wide seq tiles
    ML = NHL * DK     # local model width (q columns this core owns)

    nc = bass.Bass("TRN2", num_devices=8)
    dXT = nc.dram_tensor("XT", [D, S], F16, kind="ExternalInput")
    dWQT = nc.dram_tensor("WQT", [D, ML], F16, kind="ExternalInput")
    dWOT = nc.dram_tensor("WOT", [ML, D], F16, kind="ExternalInput")
    dQB = nc.dram_tensor("QB", [128, NHL], F32, kind="ExternalInput")
    dCOSA = nc.dram_tensor("COSA", [DK, S], F16, kind="ExternalInput")
    dSINA = nc.dram_tensor("SINA", [DK, S], F16, kind="ExternalInput")
    dTRI = nc.dram_tensor("TRI", [128, 128], F16, kind="ExternalInput")
    dONEC = nc.dram_tensor("ONEC", [128, 1], F32, kind="ExternalInput")
    dONER = nc.dram_tensor("ONER", [1, 128], F32, kind="ExternalInput")
    dIDT = nc.dram_tensor("IDT", [128, 128], F16, kind="ExternalInput")
    dOUT = nc.dram_tensor("OUTP", [S, D], F16, kind="ExternalOutput")
    if debug:
        dDBG_Q = nc.dram_tensor("DBG_Q", [DK, S], F32, kind="ExternalOutput")
        dDBG_R = nc.dram_tensor("DBG_R", [1, S], F32, kind="ExternalOutput")
        dDBG_S = nc.dram_tensor("DBG_S", [1, S], F32, kind="ExternalOutput")
        dDBG_M = nc.dram_tensor("DBG_M", [128, S // 128], F32,
                                kind="ExternalOutput")

    with tile.TileContext(nc) as tc, ExitStack() as ctx:
        pp = ctx.enter_context(tc.tile_pool(name="pp", bufs=1))

        # ---- constants (whole-kernel scope) ------------------------------
        tri = pp.tile([128, 128], BF16, name="tri")
        onecf = pp.tile([128, 1], F16, name="onecf")
        onecb = pp.tile([128, 1], BF16, name="onecb")
        oner16 = pp.tile([1, 128], F16, name="oner16")
        onerB = pp.tile([1, 128], BF16, name="onerB")
        idt = pp.tile([128, 128], F16, name="idt")
        qb = pp.tile([128, NHL], F32, name="qb")
        shift_t = pp.tile([128, 1], F32, name="shift_t")
        eps_t = pp.tile([128, 1], F32, name="eps_t")
        nc.gpsimd.dma_start(out=tri, in_=dTRI[:, :])
        nc.gpsimd.dma_start(out=onecf, in_=dONEC[:, :])
        nc.gpsimd.dma_start(out=onecb, in_=dONEC[:, :])
        nc.gpsimd.dma_start(out=oner16, in_=dONER[:, :])
        nc.gpsimd.dma_start(out=onerB, in_=dONER[:, :])
        nc.gpsimd.dma_start(out=idt, in_=dIDT[:, :])
        nc.gpsimd.dma_start(out=qb, in_=dQB[:, :])
        nc.vector.memset(shift_t, -SHIFT)
        nc.vector.memset(eps_t, EPS)

        # persistent across phases: rope'd Q, V tiles, attention out, Wo w.
        # qth/cosa/sina are persistent too so the tail head's rope (DVE)
        # doesn't gate the phase-B pool allocations (SBUF reuse barrier).
        pmid = ctx.enter_context(tc.tile_pool(name="pmid", bufs=1))
        qr = [pmid.tile([DK, S], F16, name=f"qr{h}") for h in range(NHL)]
        vv = [pmid.tile([128, S], BF16, name=f"vv{h}") for h in range(NHL)]
        wo = pmid.tile([128, NHL, D], F16, name="wo")
        qth = [pmid.tile([DK, S], F16, name=f"qth{h}") for h in range(NHL)]
        cosa = pmid.tile([DK, S], F16, name="cosa")
        sina = pmid.tile([DK, S], F16, name="sina")

        # =================== Phase A: load, RMS, Q proj ===================
        with tc.tile_pool(name="pxw", bufs=1) as pxw, \
             tc.tile_pool(name="psQ", bufs=3, space="PSUM") as psQ, \
             tc.tile_pool(name="psRow", bufs=4, space="PSUM") as psRow, \
             tc.tile_pool(name="psAux", bufs=1, space="PSUM") as psAux:

            xt = [pxw.tile([128, S], F16, name=f"xt{k}") for k in range(KT)]
            wq = pxw.tile([128, KT, ML], F16, name="wq")

            for k in range(KT):
                nc.sync.dma_start(out=xt[k], in_=dXT[k * 128:(k + 1) * 128, :])
                nc.sync.dma_start(out=wq[:, k, :],
                                  in_=dWQT[k * 128:(k + 1) * 128, :])

            # Q-proj combos; first wave overlaps the ssq/DMA window
            combos = [(mt, c) for mt in range(NHL) for c in range(SK)]
            wave0, rest = combos[:3], combos[3:]
            q_ps = {}
            for mt, c in wave0:
                q_ps[(mt, c)] = psQ.tile([128, 512], F32,
                                         name=f"qps{mt}_{c}", tag="q")

            # ssq rows: 4 x [1,512] PSUM rows (one bank each)
            ssq_ps = [psRow.tile([1, 512], F32, name=f"ssq{c}", tag="row")
                      for c in range(SK)]
            for k in range(KT):
                sq = pxw.tile([128, S], F16, name=f"sq{k}", tag="sq", bufs=2)
                nc.vector.tensor_mul(sq, xt[k], xt[k])
                for mt, c in wave0:
                    nc.tensor.matmul(
                        q_ps[(mt, c)],
                        wq[:, k, mt * 128:(mt + 1) * 128],
                        xt[k][:, c * 512:(c + 1) * 512],
                        start=(k == 0), stop=(k == KT - 1))
                for c in range(SK):
                    nc.tensor.matmul(ssq_ps[c], onecf,
                                     sq[:, c * 512:(c + 1) * 512],
                                     start=(k == 0), stop=(k == KT - 1))
            # bulk loads needed later: same queue as xt/wq so their
            # transfers don't steal HBM bandwidth from the critical xt path
            nc.sync.dma_start(out=cosa, in_=dCOSA[:, :])
            nc.sync.dma_start(out=sina, in_=dSINA[:, :])
            for h in range(NHL):
                nc.sync.dma_start(out=wo[:, h, :],
                                  in_=dWOT[h * 128:(h + 1) * 128, :])

            # rms chain: rows -> [128,16] -> rsqrt -> back to rows -> Rbc
            sqrow = pxw.tile([1, S], F32, name="sqrow")
            for c in range(SK):
                nc.vector.tensor_copy(sqrow[0:1, c * 512:(c + 1) * 512],
                                      ssq_ps[c])
            onef32 = pxw.tile([128, 1], F32, name="onef32")
            nc.gpsimd.dma_start(out=onef32, in_=dONEC[:, :])
            auxa = psAux.tile([128, 512], F32, name="auxa", tag="aux")
            for c in range(SK):
                for j in range(4):
                    t = c * 4 + j
                    nc.tensor.transpose(
                        auxa[:, t:t + 1],
                        sqrow[0:1, c * 512 + j * 128:c * 512 + (j + 1) * 128],
                        onef32[0:1, 0:1])
            ms128 = pxw.tile([128, ST], F32, name="ms128")
            nc.vector.tensor_copy(ms128, auxa[:, 0:ST])
            # r = 1/sqrt(ms/D + eps) = exp(-0.5 * ln(ms/D + eps))
            lnms = pxw.tile([128, ST], F32, name="lnms")
            nc.scalar.activation(lnms, ms128,
                                 mybir.ActivationFunctionType.Ln,
                                 bias=eps_t, scale=1.0 / D)
            r128 = pxw.tile([128, ST], F16, name="r128")
            nc.scalar.activation(r128, lnms,
                                 mybir.ActivationFunctionType.Exp,
                                 scale=-0.5)
            # transpose back column-by-column into a partition-0 row
            rrow = pxw.tile([1, S], F16, name="rrow")
            auxb = psAux.tile([128, 1024], F16, name="auxb", tag="aux")
            for c in range(SK):
                for j in range(4):
                    t = c * 4 + j
                    nc.tensor.transpose(
                        auxb[0:1, j * 128:(j + 1) * 128],
                        r128[:, t:t + 1], idt)
                nc.vector.tensor_copy(rrow[0:1, c * 512:(c + 1) * 512],
                                      auxb[0:1, 0:512])

            # Rbc: broadcast r across partitions, [128, S] f16 in SBUF
            Rbc = pxw.tile([128, S], F16, name="Rbc")
            for c in range(SK):
                rbc_ps = psRow.tile([128, 512], F32, name=f"rbc{c}", tag="row")
                for j in range(4):
                    sl = slice(c * 512 + j * 128, c * 512 + (j + 1) * 128)
                    nc.tensor.matmul(rbc_ps[:, j * 128:(j + 1) * 128], oner16,
                                     rrow[0:1, sl],
                                     start=True, stop=True)
                nc.vector.tensor_copy(Rbc[:, c * 512:(c + 1) * 512], rbc_ps)

            # Q projection drains + remaining combos; rope + V per done head
            def drain_combo(mt, c):
                sl = slice(c * 512, (c + 1) * 512)
                nc.vector.tensor_mul(qth[mt][:, sl], q_ps[(mt, c)], Rbc[:, sl])
                nc.vector.tensor_scalar_add(qth[mt][:, sl], qth[mt][:, sl],
                                            qb[:, mt:mt + 1])

            def finish_head(mt):
                # V = qth.T per head (PE transpose via identity); first so
                # the psRow banks free before the rope runs on DVE
                for g in range(ST // 4):
                    vaux = psRow.tile([128, 512], F16, name=f"vx{mt}_{g}",
                                      tag="row")
                    for j in range(4):
                        st = g * 4 + j
                        nc.tensor.transpose(
                            vaux[:, j * 128:(j + 1) * 128],
                            qth[mt][:, st * 128:(st + 1) * 128], idt)
                    nc.vector.tensor_copy(vv[mt][:, g * 512:(g + 1) * 512],
                                          vaux)
                # rope for this head (reads persistent tiles only)
                sh = pmid.tile([DK, S], F16, name=f"sh{mt}", tag="ropesh",
                               bufs=2)
                hw = DK // 2
                nc.vector.tensor_copy(sh[0:hw, :], qth[mt][hw:DK, :])
                nc.vector.tensor_copy(sh[hw:DK, :], qth[mt][0:hw, :])
                m1 = pmid.tile([DK, S], F16, name=f"m1_{mt}", tag="ropem1",
                               bufs=2)
                nc.vector.tensor_mul(m1, qth[mt], cosa)
                nc.vector.tensor_mul(sh, sh, sina)
                nc.vector.tensor_add(qr[mt], m1, sh)

            for mt, c in wave0:
                drain_combo(mt, c)
            for mt, c in rest:
                q_ps[(mt, c)] = psQ.tile([128, 512], F32,
                                         name=f"qps{mt}_{c}", tag="q")
                for k in range(KT):
                    nc.tensor.matmul(
                        q_ps[(mt, c)],
                        wq[:, k, mt * 128:(mt + 1) * 128],
                        xt[k][:, c * 512:(c + 1) * 512],
                        start=(k == 0), stop=(k == KT - 1))
                drain_combo(mt, c)
                if c == SK - 1:
                    finish_head(mt)

            if debug:
                nc.gpsimd.dma_start(out=dDBG_Q[:, :], in_=qth[0][:, :])
                nc.gpsimd.dma_start(out=dDBG_R[:, :], in_=rrow[:, :])
                nc.gpsimd.dma_start(out=dDBG_S[:, :], in_=sqrow[:, :])
                nc.gpsimd.dma_start(out=dDBG_M[:, :], in_=ms128[:, :])

        # =================== Phase B: attention + Wo ======================
        with tc.tile_pool(name="patt", bufs=1) as pat, \
             tc.tile_pool(name="psS", bufs=3, space="PSUM") as psS, \
             tc.tile_pool(name="psO", bufs=2, space="PSUM") as psO, \
             tc.tile_pool(name="psDn", bufs=1, space="PSUM") as psDn, \
             tc.tile_pool(name="psW", bufs=2, space="PSUM") as psW:

            ot = [pat.tile([DK, S], F16, name=f"ot{h}") for h in range(NHL)]
            NC = D // 512

            # per-block state carried between pipeline stages
            def new_block(h, qc):
                npt = min(ST, qc * 4 + 4)
                return {
                    "h": h, "qc": qc, "npt": npt,
                    "atq": [pat.tile([128, 512], BF16, name=f"at{h}_{qc}_{p}",
                                     tag=f"atq{p}", bufs=3)
                            for p in range(npt)],
                    "dn_ps": None,
                    "o_ps": None,
                }

            def emit_score(blk, pt):
                h, qc = blk["h"], blk["qc"]
                off = pt * 128 - qc * 512
                lo = max(0, off)
                n = 512 - lo
                s_ps = psS.tile([128, 512], F32, name=f"sps{h}_{qc}_{pt}",
                                tag="s")
                nc.tensor.matmul(
                    s_ps[:, 0:n],
                    qr[h][:, pt * 128:(pt + 1) * 128],
                    qr[h][:, qc * 512 + lo:(qc + 1) * 512],
                    start=True, stop=True)
                nc.scalar.activation(blk["atq"][pt][:, lo:512], s_ps[:, 0:n],
                                     mybir.ActivationFunctionType.Exp,
                                     bias=shift_t)
                if off >= 0:
                    # diagonal block: triangular causal mask (keep p <= q)
                    nc.vector.tensor_mul(blk["atq"][pt][:, lo:lo + 128],
                                         blk["atq"][pt][:, lo:lo + 128], tri)

            def emit_dn(blk, pt):
                off = pt * 128 - blk["qc"] * 512
                lo = max(0, off)
                if blk["dn_ps"] is None:
                    blk["dn_ps"] = psDn.tile([1, 512], F32,
                                             name=f"dn{blk['h']}_{blk['qc']}",
                                             tag="dn")
                nc.tensor.matmul(blk["dn_ps"][0:1, lo:512],
                                 onecb, blk["atq"][pt][:, lo:512],
                                 start=(pt == 0), stop=(pt == blk["npt"] - 1),
                                 skip_group_check=True)

            def emit_av(blk, pt):
                h, qc = blk["h"], blk["qc"]
                off = pt * 128 - qc * 512
                lo = max(0, off)
                if blk["o_ps"] is None:
                    blk["o_ps"] = psO.tile([128, 512], F32, name=f"o{h}_{qc}",
                                           tag="o")
                nc.tensor.matmul(blk["o_ps"][:, lo:512],
                                 vv[h][:, pt * 128:(pt + 1) * 128],
                                 blk["atq"][pt][:, lo:512],
                                 start=(pt == 0), stop=(pt == blk["npt"] - 1),
                                 skip_group_check=True)

            def emit_recip(blk):
                h, qc = blk["h"], blk["qc"]
                # 1/dn = exp(-ln(dn)) on the Act engine (PSUM read, same
                # table set as the attention exp)
                lnrow = pat.tile([1, 512], F32, name=f"lnr{h}_{qc}",
                                 tag="lnrow", bufs=2)
                nc.scalar.activation(lnrow, blk["dn_ps"],
                                     mybir.ActivationFunctionType.Ln)
                rdrow_b = pat.tile([1, 512], BF16, name=f"rdh{h}_{qc}",
                                   tag="rdrow16", bufs=2)
                nc.scalar.activation(rdrow_b, lnrow,
                                     mybir.ActivationFunctionType.Exp,
                                     scale=-1.0)
                bc2 = psW.tile([128, 512], F32, name=f"bc2{h}_{qc}", tag="w")
                nc.tensor.matmul(bc2, onerB, rdrow_b,
                                 start=True, stop=True)
                blk["bc2"] = bc2

            def emit_norm2(blk):
                h, qc = blk["h"], blk["qc"]
                rdb = pat.tile([128, 512], F32, name=f"rdb{h}_{qc}",
                               tag="rdb", bufs=2)
                nc.vector.tensor_copy(rdb, blk["bc2"])
                sl = slice(qc * 512, (qc + 1) * 512)
                nc.vector.tensor_mul(ot[h][:, sl], blk["o_ps"], rdb)

            # Wo work queue: one st-group = 4 ncc PSUM groups + copies + DMA.
            # Copies alternate DVE/Act and the output DMA goes in halves so
            # the drain tail stays short.
            def emit_wo_group(st, engine_mix=False):
                out_sb = pat.tile([128, D], F16, name=f"osb{st}", tag="osb",
                                  bufs=2)
                for ncc in range(NC):
                    wo_ps = psW.tile([128, 512], F32, name=f"wops{st}_{ncc}",
                                     tag="w")
                    for h in range(NHL):
                        nc.tensor.matmul(
                            wo_ps,
                            ot[h][:, st * 128:(st + 1) * 128],
                            wo[:, h, ncc * 512:(ncc + 1) * 512],
                            start=(h == 0), stop=(h == NHL - 1))
                    csl = slice(ncc * 512, (ncc + 1) * 512)
                    if engine_mix and ncc % 2 == 1:
                        nc.scalar.copy(out_sb[:, csl], wo_ps)
                    else:
                        nc.vector.tensor_copy(out_sb[:, csl], wo_ps)
                    if ncc == 1:
                        nc.sync.dma_start(
                            out=dOUT[st * 128:(st + 1) * 128, 0:1024],
                            in_=out_sb[:, 0:1024])
                nc.sync.dma_start(out=dOUT[st * 128:(st + 1) * 128, 1024:D],
                                  in_=out_sb[:, 1024:D])

            # depth-2 software pipeline: scores/exp of block i interleaved
            # with dn/AV of block i-2, so PE never waits on the Act engine
            prev = prev2 = None
            wo_pending = []   # st indices ready to emit
            for qc in range(SK):
                for h in range(NHL):
                    cur = new_block(h, qc)
                    np_c = cur["npt"]
                    np_p = prev2["npt"] if prev2 else 0
                    n = max(np_c, np_p)
                    for i in range(n):
                        if i < np_c:
                            emit_score(cur, i)
                        if prev2 and i < np_p:
                            emit_dn(prev2, i)
                            emit_av(prev2, i)
                            if i == np_p - 1:
                                emit_recip(prev2)
                        # sprinkle one Wo group every ~4 iterations
                        if wo_pending and i % 4 == 3:
                            emit_wo_group(wo_pending.pop(0))
                    if prev2:
                        emit_norm2(prev2)
                        if prev2["h"] == NHL - 1:
                            # all heads of prev2's qc normalized -> Wo ready
                            wo_pending.extend(range(prev2["qc"] * 4,
                                                    prev2["qc"] * 4 + 4))
                    if wo_pending:
                        emit_wo_group(wo_pending.pop(0))
                    prev2, prev = prev, cur

            # drain the two in-flight blocks + remaining Wo groups
            for blk in (prev2, prev):
                for i in range(blk["npt"]):
                    emit_dn(blk, i)
                    emit_av(blk, i)
                    if wo_pending and i % 5 == 4:
                        emit_wo_group(wo_pending.pop(0))
                emit_recip(blk)
                emit_norm2(blk)
                if blk["h"] == NHL - 1:
                    wo_pending.extend(range(blk["qc"] * 4, blk["qc"] * 4 + 4))
            while wo_pending:
                emit_wo_group(wo_pending.pop(0), engine_mix=True)

    return nc


# ======================= host-side preparation ===========================

def host_prep(X, Wq_w, Wq_b, Wo_w, Wo_b, rms_w, n_cores=8, NHL=4):
    """Build per-core input maps. X: (B,S,D) fp32."""
    B, S, D = X.shape
    DK = 128
    c = DK ** -0.25
    inv = 1.0 / (ROPE_BASE ** (np.arange(0, DK, 2, dtype=np.float64) / DK))
    ang = np.arange(S, dtype=np.float64)[:, None] * inv[None, :]
    cos = np.concatenate([np.cos(ang), np.cos(ang)], -1)     # (S, DK)
    sin = np.concatenate([np.sin(ang), np.sin(ang)], -1)
    COSA = (cos.T * c).astype(np.float16)                    # (DK, S)
    SINT = (sin.T * c).astype(np.float32)
    SINA = np.concatenate([-SINT[:DK // 2], SINT[DK // 2:]], 0).astype(np.float16)
    TRI = np.triu(np.ones((128, 128), np.float32)).astype(np.float16)  # p <= q
    ONEC = np.ones((128, 1), np.float32)
    ONER = np.ones((1, 128), np.float32)
    IDT = np.eye(128, dtype=np.float16)

    Wq_eff = (Wq_w * rms_w[None, :]).astype(np.float32)       # (D, D) fold rms
    in_maps = []
    groups = n_cores // B                                     # head-groups per batch
    ML = NHL * DK
    for core in range(n_cores):
        b = core // groups
        hg = core % groups
        msl = slice(hg * ML, (hg + 1) * ML)
        XT = np.ascontiguousarray(X[b].T).astype(np.float16)          # (D, S)
        WQT = np.ascontiguousarray(Wq_eff[msl, :].T).astype(np.float16)   # (D, ML)
        WOT = np.ascontiguousarray(Wo_w[:, msl].T).astype(np.float16)     # (ML, D)
        QB = np.ascontiguousarray(
            Wq_b[msl].reshape(NHL, 128).T).astype(np.float32)             # (128, NHL)
        in_maps.append({
            "XT": XT, "WQT": WQT, "WOT": WOT, "QB": QB,
            "COSA": COSA, "SINA": SINA, "TRI": TRI,
            "ONEC": ONEC, "ONER": ONER, "IDT": IDT,
        })
    return in_maps


def host_reduce(X, Wo_b, results, n_cores=8):
    B, S, D = X.shape
    groups = n_cores // B
    out = np.empty((B, S, D), np.float32)
    for b in range(B):
        acc = X[b].astype(np.float32).copy()
        for hg in range(groups):
            acc += results[b * groups + hg]["OUTP"].astype(np.float32)
        acc += Wo_b[None, :]
        out[b] = acc
    return out


# ======================= public entry point ==============================

_CACHE = {}


def _get_nc():
    if "nc" not in _CACHE:
        nc = build_core(S=2048, D=2048, NHL=4, DK=128, SHIFT=10.0)
        legalize_sync_waits(nc, max_waits=1)
        _CACHE["nc"] = nc
    return _CACHE["nc"]


def kernel(X, Wq_w, Wq_b, Wo_w, Wo_b, rms_w):
    """Full-input MHA block: returns X + MHA(RMSNorm(X)) as np.float32.

    Shards batch(2) x head-groups(4) across 8 NeuronCores; each core
    produces a partial output (its 4 heads through Wo); the host sums the
    four partials per batch and adds bias + residual.
    """
    from concourse.bass_utils import run_bass_kernel_spmd

    X = np.asarray(X, np.float32)
    Wq_w = np.asarray(Wq_w, np.float32)
    Wq_b = np.asarray(Wq_b, np.float32)
    Wo_w = np.asarray(Wo_w, np.float32)
    Wo_b = np.asarray(Wo_b, np.float32)
    rms_w = np.asarray(rms_w, np.float32)

    nc = _get_nc()
    in_maps = host_prep(X, Wq_w, Wq_b, Wo_w, Wo_b, rms_w)
    res = run_bass_kernel_spmd(nc, in_maps, core_ids=list(range(8)))
    return host_reduce(X, Wo_b, res.results)
